# revision 20
# baseline (speedup 1.0000x reference)
"""Lorentz-hyperboloid ViT-B (DinoVisionTransformer variant) forward pass on
8 Trainium2 NeuronCores, data-parallel over the batch (4 images / core).

Layout strategy (per core, 4 images, 197 tokens each, img-padded to 256):
  - Residual stream `tok` token-major [1024 (8x128 tiles), 768] f32, col 0 =
    time.
  - LN space-part -> PE-transpose -> xsT feature-major [768, 1024] bf16.
  - qT/kT computed feature-major bf16 (Wq/Wk columns host-permuted so rope
    pair-elements 0/1 form row blocks [0:384) / [384:768)); rope writes the
    even-output rows to a separate `rot` tile (no copy-back).
  - Lorentz scores via two K=32 bf16 matmuls per head on a 256-wide query
    block (one image), plus a K=1 matmul adding the -qt*kt time term
    (replaces per-head DMA broadcasts of qt).
  - Softmax normalization is skipped: the Lorentz projection after the
    attention midpoint is scale-invariant.
  - All sqrt/rsqrt computed as exp(+-0.5*ln(x)) so the whole layer outside
    the MLP runs off one activation table (ln+exp); the MLP uses Silu
    directly -> 2 activation-table swaps per layer.
  - Weights are bf16 and host-prepped into partition-major layouts so each
    weight block loads with a single DMA; Wv/Wo/W3 are SBUF-resident per
    layer, Wq/Wk/W1/W2 stream per 128-column block.
  - SwiGLU gate spilled to DRAM in bf16 (batched, p-major), read back with
    4 DMAs per output block.
"""
import math
import numpy as np
from contextlib import ExitStack

import ml_dtypes
import concourse.bass as bass
import concourse.tile as tile
from concourse import bacc, mybir
from concourse.bass_utils import run_bass_kernel_spmd
from concourse.masks import make_identity

F32 = mybir.dt.float32
F32R = mybir.dt.float32r
BF16 = mybir.dt.bfloat16
AF = mybir.ActivationFunctionType
ALU = mybir.AluOpType
AXX = mybir.AxisListType.X

B, IMG, PS, CIN, D, H, L = 32, 224, 16, 3, 768, 12, 12
HD, DFF, C, EPS = 64, 2048, 1.0, 1e-6
N = 197
NC_CORES = 8
BC = B // NC_CORES
NP = 256                 # padded tokens per image
T = BC * NP              # 1024
TM = T // 128            # 8 token tiles
KD = D // 128            # 6 feature tiles
SCALE = math.sqrt(768.0)
NCH = (384, 383)
NOFF = (0, 384)


def _rope_tables():
    n = IMG // PS
    d4 = HD // 4
    inv = 1.0 / (100.0 ** (np.arange(d4) / d4))
    ang = np.arange(n)[:, None] * inv[None, :]
    ay = np.repeat(ang[:, None, :], n, axis=1)
    ax = np.repeat(ang[None, :, :], n, axis=0)
    a = np.concatenate([ay, ax], -1).reshape(n * n, HD // 2)
    cos = np.concatenate([np.ones((1, HD // 2)), np.cos(a)], 0)
    sin = np.concatenate([np.zeros((1, HD // 2)), np.sin(a)], 0)
    return cos.astype(np.float32), sin.astype(np.float32)


def _pmajor(w, kd):
    """[L, kd*128, E] -> [L, 128, kd*E] partition-major blocks."""
    l, k, e = w.shape
    assert k == kd * 128
    return np.ascontiguousarray(
        w.reshape(l, kd, 128, e).transpose(0, 2, 1, 3).reshape(l, 128, kd * e))


def host_prep(inputs):
    ins = {k: np.ascontiguousarray(np.asarray(v)) for k, v in inputs.items()}
    f32 = np.float32
    bf16 = ml_dtypes.bfloat16

    cos, sin = _rope_tables()
    cosP = np.zeros((128, T), f32)
    sinP = np.zeros((128, T), f32)
    for b in range(BC):
        for rep in range(4):
            cosP[rep * 32:(rep + 1) * 32, b * NP:b * NP + N] = cos.T
            sinP[rep * 32:(rep + 1) * 32, b * NP:b * NP + N] = sin.T

    perm = np.zeros(768, dtype=np.int64)
    for h in range(H):
        for i in range(32):
            perm[h * 32 + i] = h * 64 + 2 * i
            perm[384 + h * 32 + i] = h * 64 + 2 * i + 1

    g1 = ins['ln1_g'][:, :, None]
    b1 = ins['ln1_b']
    g2 = ins['ln2_g'][:, :, None]
    b2 = ins['ln2_b']

    def padrow(w):
        z = np.zeros((w.shape[0], 1, w.shape[2]), f32)
        return np.concatenate([w, z], 1)

    Wq = padrow(g1 * ins['Wq'][:, :, perm])
    Wk = padrow(g1 * ins['Wk'][:, :, perm])
    Wv = padrow(g1 * ins['Wv'])
    bq = np.einsum('ld,lde->le', b1, ins['Wq'][:, :, perm]).astype(f32)
    bk = np.einsum('ld,lde->le', b1, ins['Wk'][:, :, perm]).astype(f32)
    bv = np.einsum('ld,lde->le', b1, ins['Wv']).astype(f32)
    W1 = padrow(g2 * ins['W1'])
    W2 = padrow(g2 * ins['W2'])
    b1m = np.einsum('ld,lde->le', b2, ins['W1']).astype(f32)
    b2m = np.einsum('ld,lde->le', b2, ins['W2']).astype(f32)
    Wo = np.concatenate([ins['Wo'], np.zeros((L, 768, 1), f32)], 2)
    W3 = np.concatenate([ins['W3'], np.zeros((L, 2048, 1), f32)], 2)

    def _mmajor(w):
        # [L, 128, KD*768] -> [L, 6, 128, KD*128] per-output-block layout
        return np.ascontiguousarray(
            w.reshape(L, 128, KD, KD, 128).transpose(0, 3, 1, 2, 4)
            .reshape(L, KD, 128, KD * 128))

    # partition-major bf16 weight layouts (single-DMA loads)
    WqP = _mmajor(_pmajor(Wq, KD)).astype(bf16)
    WkP = _mmajor(_pmajor(Wk, KD)).astype(bf16)
    WvP = _pmajor(Wv, KD).astype(bf16)
    WoP = _pmajor(Wo, KD).astype(bf16)
    # W1/W2: m-major: [L, 16, 128, 6*128]
    W1P = np.ascontiguousarray(
        _pmajor(W1, KD).reshape(L, 128, KD, 16, 128)
        .transpose(0, 3, 1, 2, 4).reshape(L, 16, 128, KD * 128)).astype(bf16)
    W2P = np.ascontiguousarray(
        _pmajor(W2, KD).reshape(L, 128, KD, 16, 128)
        .transpose(0, 3, 1, 2, 4).reshape(L, 16, 128, KD * 128)).astype(bf16)
    # W3: [L, 2, 128, 16*384] halves
    W3p = _pmajor(W3, 16).reshape(L, 128, 16, 768)
    W3P = np.ascontiguousarray(np.stack(
        [W3p[:, :, :, 0:384], W3p[:, :, :, 384:768]], 1)
        .reshape(L, 2, 128, 16 * 384)).astype(bf16)

    Wpr = ins['Wp'].transpose(2, 0, 1, 3).reshape(1024, 767).astype(f32)
    Wpr = np.concatenate([Wpr, np.zeros((1024, 1), f32)], 1)
    WprP = np.ascontiguousarray(
        Wpr.reshape(8, 128, 768).transpose(1, 0, 2)
        .reshape(128, 8 * 768)).astype(bf16)

    cls_s = ins['cls_s']
    cls_vec = np.concatenate(
        [np.sqrt((cls_s ** 2).sum(keepdims=True) + C), cls_s]).astype(f32)

    # head-sum matmul: row k -> output partition 32*(k//32) (band base);
    # all other output partitions stay zero.
    E4 = np.zeros((128, 128), f32)
    for k in range(128):
        E4[k, 32 * (k // 32)] = 1.0

    # per-partition bias tables [128, L*cols]
    bqP = np.ascontiguousarray(
        bq.reshape(L, KD, 128).transpose(2, 0, 1).reshape(128, L * KD))
    bkP = np.ascontiguousarray(
        bk.reshape(L, KD, 128).transpose(2, 0, 1).reshape(128, L * KD))
    b1P = np.ascontiguousarray(
        b1m.reshape(L, 16, 128).transpose(2, 0, 1).reshape(128, L * 16))
    b2P = np.ascontiguousarray(
        b2m.reshape(L, 16, 128).transpose(2, 0, 1).reshape(128, L * 16))

    return {
        'WqP': WqP, 'WkP': WkP, 'WvP': WvP, 'WoP': WoP,
        'W1P': W1P, 'W2P': W2P, 'W3P': W3P,
        'bqP': bqP, 'bkP': bkP, 'b1P': b1P, 'b2P': b2P,
        'bv': bv,
        'WprP': WprP, 'cls': cls_vec.reshape(1, 768),
        'cosP': cosP.astype(bf16), 'sinP': sinP.astype(bf16),
        'E4': E4.astype(bf16),
        'wy1': ins['wy1'].astype(f32).reshape(1, L),
        'wy2': ins['wy2'].astype(f32).reshape(1, L),
        'lnf_g': ins['lnf_g'].astype(f32).reshape(1, 767),
        'lnf_b': ins['lnf_b'].astype(f32).reshape(1, 767),
    }


def core_input(x_full, core):
    f32 = np.float32
    xs = np.asarray(x_full[core * BC:(core + 1) * BC])
    n = IMG // PS
    xp = xs.reshape(BC, 3, n, PS, n, PS).transpose(0, 2, 4, 1, 3, 5)
    xp = xp.reshape(BC, n * n, 3, PS * PS)
    xpp = np.zeros((768, T), f32)
    for b in range(BC):
        cols = b * NP + 1 + np.arange(n * n)
        xpp[:, cols] = xp[b].transpose(1, 2, 0).reshape(768, n * n)
    # partition-major [128, 6, T]
    return np.ascontiguousarray(
        xpp.reshape(6, 128, T).transpose(1, 0, 2)
        .reshape(128, 6 * T)).astype(ml_dtypes.bfloat16)


# ======================================================================
# device program
# ======================================================================

def build_program(n_layers=L, final_ln=True):
    nc = bacc.Bacc("TRN2", target_bir_lowering=False, debug=False,
                   num_devices=NC_CORES)
    dp = nc.declare_dram_parameter
    d = {}
    for nm, sh, dt in [
            ('xpp', [128, KD * T], BF16),
            ('WqP', [L, KD, 128, KD * 128], BF16),
            ('WkP', [L, KD, 128, KD * 128], BF16),
            ('WvP', [L, 128, KD * 768], BF16), ('WoP', [L, 128, KD * 768], BF16),
            ('W1P', [L, 16, 128, KD * 128], BF16),
            ('W2P', [L, 16, 128, KD * 128], BF16),
            ('W3P', [L, 2, 128, 16 * 384], BF16),
            ('bqP', [128, L * KD], F32), ('bkP', [128, L * KD], F32),
            ('b1P', [128, L * 16], F32), ('b2P', [128, L * 16], F32),
            ('bv', [L, 768], F32),
            ('WprP', [128, 8 * 768], BF16), ('cls', [1, 768], F32),
            ('cosP', [128, T], BF16), ('sinP', [128, T], BF16),
            ('E4', [128, 128], BF16),
            ('wy1', [1, L], F32), ('wy2', [1, L], F32),
            ('lnf_g', [1, 767], F32), ('lnf_b', [1, 767], F32)]:
        d[nm] = dp(nm, sh, dt, isOutput=False).ap()
    d['out'] = dp('out', [BC * N, 768], F32, isOutput=True).ap()

    with tile.TileContext(nc) as tc, ExitStack() as ctx:
        Prog(ctx, tc, d).run(n_layers, final_ln)
    nc.compile()
    return nc


class Prog:
    def __init__(self, ctx, tc, d):
        self.tc, self.nc, self.d = tc, tc.nc, d
        p = lambda name, bufs, space='SBUF': ctx.enter_context(
            tc.tile_pool(name=name, bufs=bufs, space=space))
        self.singles = p('singles', 1)
        self.fm = p('fm', 2)          # xsT / msT / m_space / xsT2  (bf16 12K)
        self.qkp = p('qkp', 2)        # qT / kT bf16
        self.rotp = p('rotp', 2)      # rope even-row outputs [128,3,T] bf16
        self.wres = p('wres', 2)      # Wv / Wo resident [128,KD,768] bf16
        self.w3res = p('w3res', 2)    # W3 halves [128,16,384] bf16
        self.w12 = p('w12', 6)        # streamed [128,KD,128] bf16 blocks
        self.gwp = p('gwp', 2)        # g write tiles [128,T] bf16
        self.gtp = p('gtp', 3)        # g read tiles [128,4,512] bf16
        self.ptp = p('ptp', 3)        # exp(scores) [128,2,256] bf16
        self.vfp = p('vfp', 5)        # vf [128,2,12,66] bf16
        self.h12 = p('h12', 3)        # [128,T] bf16 scratch
        self.amp = p('amp', 5)        # a/m token tiles [128,768] f32
        self.smp = p('smp', 2)        # bvs
        self.qtp = p('qtp', 4)        # qt/ktn rows [12,T] bf16
        self.s2p = p('s2p', 8)        # [128,16] per-partition scalars
        self.lnfp = p('lnfp', 1)      # final LN gain/bias
        self.dramp = p('dramp', 1, 'DRAM')
        self.acc = p('acc', 4, 'PSUM')    # [128,512] accumulators
        self.mm = p('mm', 2, 'PSUM')      # scores / head-sum psums
        self.tpp = p('tpp', 2, 'PSUM')    # [128,128] transposes

        nc = self.nc
        s = self.singles
        self.tok = s.tile([128, TM, 768], F32)
        self.cos_s = s.tile([128, T], BF16)
        self.sin_s = s.tile([128, T], BF16)
        self.ident = s.tile([128, 128], BF16)
        self.E4_s = s.tile([128, 128], BF16)
        self.qmask = s.tile([128, 1], F32)
        self.kmask = s.tile([128, 1], F32)
        self.wy1_s = s.tile([128, L], F32)
        self.wy2_s = s.tile([128, L], F32)
        self.bqs = s.tile([128, L, KD], F32)
        self.bks = s.tile([128, L, KD], F32)
        self.b1s = s.tile([128, L, 16], F32)
        self.b2s = s.tile([128, L, 16], F32)
        self.eps_s = s.tile([128, 1], F32)
        self.lneps = s.tile([128, 1], F32)
        self.expb = s.tile([128, 1], F32)
        nc.vector.memset(self.lneps, 1e-6)
        nc.vector.memset(self.expb, 2.0 * C / SCALE)
        nc.vector.memset(self.eps_s, EPS)
        nc.vector.memset(self.qmask, 0.0)
        nc.vector.memset(self.kmask, 0.0)
        for r in (0, 32, 64, 96):
            nc.vector.memset(self.qmask[r:r + 1, :], 1.0)
            nc.vector.memset(self.kmask[r:r + 1, :], -1.0)
        nc.sync.dma_start(out=self.cos_s, in_=d['cosP'])
        nc.sync.dma_start(out=self.sin_s, in_=d['sinP'])
        nc.sync.dma_start(out=self.E4_s, in_=d['E4'])
        nc.sync.dma_start(out=self.wy1_s, in_=d['wy1'].partition_broadcast(128))
        nc.sync.dma_start(out=self.wy2_s, in_=d['wy2'].partition_broadcast(128))
        nc.sync.dma_start(out=self.bqs,
                          in_=d['bqP'].rearrange('p (l a) -> p l a', a=KD))
        nc.sync.dma_start(out=self.bks,
                          in_=d['bkP'].rearrange('p (l a) -> p l a', a=KD))
        nc.sync.dma_start(out=self.b1s,
                          in_=d['b1P'].rearrange('p (l a) -> p l a', a=16))
        nc.sync.dma_start(out=self.b2s,
                          in_=d['b2P'].rearrange('p (l a) -> p l a', a=16))
        make_identity(nc, self.ident)
        self.gdram = [self.dramp.tile([128, 16, T], BF16, name=f'gdram{i}')
                      for i in range(2)]

    # ---------------- helpers ----------------
    def pe_T(self, dst, src):
        """PE transpose src [128, w<=128] bf16 -> dst [w, 128] via psum."""
        w = src.shape[-1]
        ps = self.tpp.tile([128, 128], BF16, tag='tp')
        self.nc.tensor.transpose(ps[:w, :], src, self.ident)
        self.nc.vector.tensor_copy(out=dst, in_=ps[:w, :])

    def sqrt_ln(self, out, in_, bias, scale=0.5, pre=1.0, tmp=None):
        """out = (pre*in_+bias)^(2*scale) via exp(scale*ln(pre*x+bias)).
        The ln intermediate lands in `tmp` (f32; defaults to in_, which is
        clobbered)."""
        nc = self.nc
        if tmp is None:
            tmp = in_
        nc.scalar.activation(out=tmp, in_=in_, func=AF.Ln, bias=bias,
                             scale=pre)
        nc.scalar.activation(out=out, in_=tmp, func=AF.Exp, scale=scale)

    def ln_xsn(self, t):
        """LN (no gain/bias) over space part of tok tile t -> xsn [128,T]
        bf16 (cols 0:767 valid)."""
        nc = self.nc
        xs = self.tok[:, t, 1:768]
        scr = self.h12.tile([128, T], BF16, tag='h12')
        s2 = self.s2p.tile([128, 4], F32, tag='s2')
        nc.vector.tensor_tensor_reduce(
            out=scr[:, :767], in0=xs, in1=xs, scale=1.0, scalar=0.0,
            op0=ALU.mult, op1=ALU.add, accum_out=s2[:, 0:1])
        nc.vector.reduce_sum(out=s2[:, 1:2], in_=xs, axis=AXX)
        nc.scalar.mul(out=s2[:, 2:3], in_=s2[:, 1:2], mul=1.0 / 767.0)
        nc.vector.tensor_mul(out=s2[:, 3:4], in0=s2[:, 1:2], in1=s2[:, 2:3])
        nc.vector.tensor_sub(out=s2[:, 3:4], in0=s2[:, 0:1], in1=s2[:, 3:4])
        # rstd = exp(-0.5 * ln(var + eps))
        nc.scalar.activation(out=s2[:, 3:4], in_=s2[:, 3:4], func=AF.Ln,
                             bias=self.lneps[:, 0:1], scale=1.0 / 767.0)
        nc.scalar.activation(out=s2[:, 3:4], in_=s2[:, 3:4], func=AF.Exp,
                             scale=-0.5)
        xsn = self.h12.tile([128, T], BF16, tag='h12')
        nc.vector.tensor_scalar(out=xsn[:, :767], in0=xs,
                                scalar1=s2[:, 2:3], scalar2=s2[:, 3:4],
                                op0=ALU.subtract, op1=ALU.mult)
        return xsn

    def make_xsT(self):
        xsT = self.fm.tile([128, KD, T], BF16, tag='fm')
        self.nc.vector.memset(xsT[:, 5, :], 0.0)
        for t in range(TM):
            xsn = self.ln_xsn(t)
            for c in range(KD):
                w = min(128, 767 - c * 128)
                self.pe_T(xsT[:w, c, t * 128:(t + 1) * 128],
                          xsn[:, c * 128:c * 128 + w])
        return xsT

    def residual(self, wy_s, l, a_t, t):
        """tok[:,t] = project(tok[:,t] + wy[l] * a) with a = a_t [128,768]
        (space in cols 1:768); computes a's time col first."""
        nc = self.nc
        scr = self.h12.tile([128, T], BF16, tag='h12')
        s2 = self.s2p.tile([128, 4], F32, tag='s2')
        nc.vector.tensor_tensor_reduce(
            out=scr[:, :767], in0=a_t[:, 1:768], in1=a_t[:, 1:768],
            scale=1.0, scalar=0.0, op0=ALU.mult, op1=ALU.add,
            accum_out=s2[:, 0:1])
        self.sqrt_ln(a_t[:, 0:1], s2[:, 0:1], C, 0.5)
        tokt = self.tok[:, t, :]
        nc.vector.scalar_tensor_tensor(
            out=tokt, in0=a_t, scalar=wy_s[:, l:l + 1], in1=tokt,
            op0=ALU.mult, op1=ALU.add)
        nc.vector.tensor_tensor_reduce(
            out=scr[:, :768], in0=tokt, in1=tokt, scale=1.0, scalar=0.0,
            op0=ALU.mult, op1=ALU.add, accum_out=s2[:, 1:2])
        nc.vector.tensor_mul(out=s2[:, 2:3], in0=tokt[:, 0:1],
                             in1=tokt[:, 0:1])
        nc.vector.scalar_tensor_tensor(
            out=s2[:, 2:3], in0=s2[:, 2:3], scalar=2.0, in1=s2[:, 1:2],
            op0=ALU.mult, op1=ALU.subtract)        # 2 t^2 - sum = -zz
        nc.vector.tensor_scalar_max(out=s2[:, 2:3], in0=s2[:, 2:3],
                                    scalar1=self.eps_s[:, 0:1])
        self.sqrt_ln(s2[:, 2:3], s2[:, 2:3], 0.0, -0.5)
        nc.vector.tensor_scalar_mul(out=tokt, in0=tokt, scalar1=s2[:, 2:3])

    # ---------------- phases ----------------
    def patch_embed(self):
        nc, d = self.nc, self.d
        wpr = self.fm.tile([128, 8, 768], BF16, tag='fm', name='wpr')
        nc.sync.dma_start(out=wpr,
                          in_=d['WprP'].rearrange('p (a e) -> p a e', e=768))
        xr = d['xpp'].rearrange('p (a t) -> p a t', t=T)
        AT_a = self.qkp.tile([128, 4, T], BF16, tag='qk')
        AT_b = self.qkp.tile([128, 4, T], BF16, tag='qk')
        nc.sync.dma_start(out=AT_a[:, 2:4, :], in_=xr[:, 0:2, :])
        nc.sync.dma_start(out=AT_b, in_=xr[:, 2:6, :])
        for pt in range(2):
            s = self.h12.tile([128, T], F32, tag='hf32')
            t2 = self.h12.tile([128, T], F32, tag='hf32')
            nc.vector.tensor_mul(out=s, in0=AT_a[:, 2 + pt, :],
                                 in1=AT_a[:, 2 + pt, :])
            nc.vector.tensor_mul(out=t2, in0=AT_b[:, pt, :],
                                 in1=AT_b[:, pt, :])
            nc.vector.tensor_add(out=s, in0=s, in1=t2)
            nc.vector.tensor_mul(out=t2, in0=AT_b[:, 2 + pt, :],
                                 in1=AT_b[:, 2 + pt, :])
            nc.vector.tensor_add(out=s, in0=s, in1=t2)
            self.sqrt_ln(AT_a[:, pt, :], s, C, 0.5)
        for g in range(2):
            for n in range(2):
                pss = [self.acc.tile([128, 512], F32, tag='acc', name=f'acc{i}')
                       for i in range(4)]
                for k in range(8):
                    src = AT_a if k < 4 else AT_b
                    for ti in range(4):
                        t = g * 4 + ti
                        nc.tensor.matmul(
                            pss[ti][:, :384],
                            (src[:, k % 4, t * 128:(t + 1) * 128]),
                            (wpr[:, k, NOFF[n]:NOFF[n] + 384]),
                            start=(k == 0), stop=(k == 7))
                for ti in range(4):
                    t = g * 4 + ti
                    nc.vector.tensor_copy(
                        out=self.tok[:, t, 1 + NOFF[n]:1 + NOFF[n] + NCH[n]],
                        in_=pss[ti][:, :NCH[n]])
        for b in range(BC):
            nc.sync.dma_start(out=self.tok[0:1, 2 * b, :], in_=d['cls'])
        for t in range(TM):
            scr = self.h12.tile([128, T], BF16, tag='h12')
            s2 = self.s2p.tile([128, 4], F32, tag='s2')
            nc.vector.tensor_tensor_reduce(
                out=scr[:, :767], in0=self.tok[:, t, 1:768],
                in1=self.tok[:, t, 1:768], scale=1.0, scalar=0.0,
                op0=ALU.mult, op1=ALU.add, accum_out=s2[:, 0:1])
            self.sqrt_ln(self.tok[:, t, 0:1], s2[:, 0:1], C, 0.5)

    def qkT_phase(self, xsT, wsrc, bias_s, l):
        """qT or kT [128, KD, T] bf16 feature-major = W^T @ xsT (+bias)."""
        nc = self.nc
        dst = self.qkp.tile([128, KD, T], BF16, tag='qk')
        for m in range(KD):
            wt = self.w12.tile([128, KD, 128], BF16, tag='w12')
            nc.sync.dma_start(
                out=wt, in_=wsrc[l, m].rearrange('p (a e) -> p a e', e=128))
            for n in range(2):
                ps = self.acc.tile([128, 512], F32, tag='acc')
                for k in range(KD):
                    nc.tensor.matmul(
                        ps, (wt[:, k, :]),
                        (xsT[:, k, n * 512:(n + 1) * 512]),
                        start=(k == 0), stop=(k == KD - 1))
                nc.scalar.activation(
                    out=dst[:, m, n * 512:(n + 1) * 512], in_=ps,
                    func=AF.Identity, bias=bias_s[:, l, m:m + 1])
        return dst

    def rope(self, zT):
        """Rotate pairs; even outputs to new rot tile, odd in place."""
        nc = self.nc
        rot = self.rotp.tile([128, 3, T], BF16, tag='rot')
        for i in range(3):
            z0 = zT[:, i, :]
            z1 = zT[:, i + 3, :]
            t0 = self.h12.tile([128, T], BF16, tag='h12')
            t1 = self.h12.tile([128, T], BF16, tag='h12')
            nc.vector.tensor_mul(out=t0, in0=z0, in1=self.cos_s)
            nc.vector.tensor_mul(out=t1, in0=z1, in1=self.sin_s)
            nc.vector.tensor_sub(out=rot[:, i, :], in0=t0, in1=t1)
            nc.vector.tensor_mul(out=t0, in0=z0, in1=self.sin_s)
            nc.vector.tensor_mul(out=t1, in0=z1, in1=self.cos_s)
            nc.vector.tensor_add(out=z1, in0=t0, in1=t1)
        return rot

    def head_time(self, zT, rot, mask):
        """Per-head Lorentz times as band-aligned tiles [128, 3, T] bf16:
        head h's time row sits at partition 32*(h%4), slot h//4; all other
        partitions zero (negated via mask for the k side)."""
        nc = self.nc
        dst = self.qtp.tile([128, 3, T], BF16, tag='qt')
        tmp = self.qtp.tile([128, T], F32, tag='qtf')
        for t in range(3):
            ps2 = [self.acc.tile([128, 512], F32, tag='acc', name=f'ht{i}')
                   for i in range(2)]
            for c in (t, t + 3):
                src = rot[:, t, :] if c < 3 else zT[:, c, :]
                sq = self.h12.tile([128, T], BF16, tag='h12')
                nc.vector.tensor_mul(out=sq, in0=src, in1=src)
                for cch in range(2):
                    nc.tensor.matmul(
                        ps2[cch], self.E4_s,
                        (sq[:, cch * 512:(cch + 1) * 512]),
                        start=(c == t), stop=(c == t + 3))
            for cch in range(2):
                sl = slice(cch * 512, (cch + 1) * 512)
                nc.scalar.activation(out=tmp[:, sl], in_=ps2[cch],
                                     func=AF.Ln, bias=C)
                nc.scalar.activation(out=dst[:, t, sl], in_=tmp[:, sl],
                                     func=AF.Exp, scale=0.5)
        nc.vector.tensor_scalar_mul(out=dst, in0=dst, scalar1=mask[:, 0:1])
        return dst

    def v_img(self, xsT, wv, bvs, b):
        """v for image b -> vf_b [128, 2, 12, 66] bf16 (+bias, +time);
        col 65 of each head is padding (never consumed)."""
        nc = self.nc
        vf = self.vfp.tile([128, 2, H, HD + 2], BF16, tag='vf')
        nc.vector.memset(vf[:, :, :, HD + 1:HD + 2], 0.0)
        for n in range(2):
            pss = [self.acc.tile([128, 512], F32, tag='acc', name=f'acc{i}')
                   for i in range(2)]
            for k in range(KD):
                for kt in range(2):
                    t = 2 * b + kt
                    nc.tensor.matmul(
                        pss[kt][:, :384],
                        (xsT[:, k, t * 128:(t + 1) * 128]),
                        (wv[:, k, n * 384:n * 384 + 384]),
                        start=(k == 0), stop=(k == KD - 1))
            for kt in range(2):
                vfv = vf[:, kt, n * 6:(n + 1) * 6, 1:HD + 1]
                psv = pss[kt][:, :384].rearrange('p (h e) -> p h e', e=HD)
                bvv = bvs[:, n * 384:(n + 1) * 384].rearrange(
                    'p (h e) -> p h e', e=HD)
                nc.vector.tensor_tensor(out=vfv, in0=psv, in1=bvv,
                                        op=ALU.add)
                sq = self.h12.tile([128, T], BF16, tag='h12')
                sqv = sq[:, :384].rearrange('p (h e) -> p h e', e=HD)
                nc.vector.tensor_tensor(out=sqv, in0=vfv, in1=vfv,
                                        op=ALU.mult)
                red = self.s2p.tile([128, 8], F32, tag='s2')
                nc.vector.reduce_sum(out=red[:, :6], in_=sqv, axis=AXX)
                self.sqrt_ln(vf[:, kt, n * 6:(n + 1) * 6, 0], red[:, :6],
                             C, 0.5)
        return vf

    def attention(self, qT, qrot, kT, krot, qt_r, ktn_r, vfs, l):
        nc = self.nc
        m_space = self.fm.tile([128, TM, 768], BF16, tag='fm')
        for b in range(BC):
            vf = vfs[b]
            qcol = b * NP
            for half in range(2):
                psA = [self.acc.tile([128, 512], F32, tag='acc', name=f'psA{i}')
                       for i in range(2)]
                for hh in range(6):
                    h = half * 6 + hh
                    r0 = (h * 32) % 128
                    c0 = h // 4
                    P_t = self.ptp.tile([128, 2, NP], BF16, tag='P')
                    for kt in range(2):
                        keys = 128 if kt == 0 else N - 128
                        ps = self.mm.tile([128, 512], F32, tag='mm')
                        kcol = b * NP + kt * 128
                        nc.tensor.matmul(
                            ps[:keys, :NP],
                            (krot[r0:r0 + 32, c0, kcol:kcol + keys]),
                            (qrot[r0:r0 + 32, c0, qcol:qcol + NP]),
                            start=True, stop=False, tile_position=(r0, 0))
                        nc.tensor.matmul(
                            ps[:keys, :NP],
                            (kT[r0:r0 + 32, c0 + 3, kcol:kcol + keys]),
                            (qT[r0:r0 + 32, c0 + 3, qcol:qcol + NP]),
                            start=False, stop=False, tile_position=(r0, 0))
                        nc.tensor.matmul(
                            ps[:keys, :NP],
                            (ktn_r[r0:r0 + 32, c0, kcol:kcol + keys]),
                            (qt_r[r0:r0 + 32, c0, qcol:qcol + NP]),
                            start=False, stop=True, tile_position=(r0, 0))
                        nc.scalar.activation(
                            out=P_t[:keys, kt, :], in_=ps[:keys, :NP],
                            func=AF.Exp, bias=self.expb[:keys, 0:1],
                            scale=2.0 / SCALE)
                    for qi in range(2):
                        qn = 128 if qi == 0 else N - 128
                        for kt in range(2):
                            keys = 128 if kt == 0 else N - 128
                            nc.tensor.matmul(
                                psA[qi][:qn, hh * 66:hh * 66 + 66],
                                P_t[:keys, kt, qi * 128:qi * 128 + qn],
                                vf[:keys, kt, h, :],
                                start=(kt == 0), stop=(kt == 1))
                for qi in range(2):
                    qn = 128 if qi == 0 else N - 128
                    psv = psA[qi][:qn, :396].rearrange('p (h e) -> p h e', e=66)
                    sq = self.h12.tile([128, T], BF16, tag='h12')
                    sqv = sq[:qn, :396].rearrange('p (h e) -> p h e', e=66)
                    nc.vector.tensor_tensor(out=sqv[:, :, 0:65],
                                            in0=psv[:, :, 0:65],
                                            in1=psv[:, :, 0:65], op=ALU.mult)
                    red = self.s2p.tile([128, 16], F32, tag='s2')
                    nc.vector.reduce_sum(out=red[:qn, :6], in_=sqv[:, :, 0:65],
                                         axis=AXX)
                    nc.vector.scalar_tensor_tensor(
                        out=red[:qn, :6], in0=sqv[:, :, 0], scalar=2.0,
                        in1=red[:qn, :6], op0=ALU.mult, op1=ALU.subtract)
                    nc.vector.tensor_scalar_max(
                        out=red[:qn, :6], in0=red[:qn, :6],
                        scalar1=self.eps_s[:qn, 0:1])
                    nc.scalar.activation(out=red[:qn, :6], in_=red[:qn, :6],
                                         func=AF.Ln)
                    nc.scalar.activation(out=red[:qn, :6], in_=red[:qn, :6],
                                         func=AF.Exp, scale=-0.5)
                    mv = m_space[:qn, 2 * b + qi,
                                 half * 384:half * 384 + 384].rearrange(
                                     'p (h e) -> p h e', e=HD)
                    nc.vector.tensor_tensor(
                        out=mv, in0=psv[:, :, 1:65],
                        in1=red[:qn, :6].broadcast_to((qn, 6, HD)),
                        op=ALU.mult)
        return m_space

    def wo_phase(self, m_space, wo, l):
        """msT = m_space^T; a = m_space @ Wo; fused residual-project."""
        nc = self.nc
        msT = self.fm.tile([128, KD, T], BF16, tag='fm')
        for t in range(TM):
            for c in range(KD):
                self.pe_T(msT[:, c, t * 128:(t + 1) * 128],
                          m_space[:, t, c * 128:(c + 1) * 128])
        for g in range(2):
            a_ts = [self.amp.tile([128, 768], F32, tag='am', name=f'a{i}')
                    for i in range(4)]
            for n in range(2):
                pss = [self.acc.tile([128, 512], F32, tag='acc', name=f'acc{i}')
                       for i in range(4)]
                for k in range(KD):
                    for ti in range(4):
                        t = g * 4 + ti
                        nc.tensor.matmul(
                            pss[ti][:, :384],
                            (msT[:, k, t * 128:(t + 1) * 128]),
                            (wo[:, k, NOFF[n]:NOFF[n] + 384]),
                            start=(k == 0), stop=(k == KD - 1))
                for ti in range(4):
                    nc.vector.tensor_copy(
                        out=a_ts[ti][:, 1 + NOFF[n]:1 + NOFF[n] + NCH[n]],
                        in_=pss[ti][:, :NCH[n]])
            for ti in range(4):
                self.residual(self.wy1_s, l, a_ts[ti], g * 4 + ti)

    def mlp_h_phase(self, xsT2, l, gdram):
        nc, d = self.nc, self.d
        for m in range(16):
            w1t = self.w12.tile([128, KD, 128], BF16, tag='w12', name='w1t')
            w2t = self.w12.tile([128, KD, 128], BF16, tag='w12', name='w2t')
            nc.sync.dma_start(
                out=w1t, in_=d['W1P'][l, m].rearrange('p (a e) -> p a e', e=128))
            nc.sync.dma_start(
                out=w2t, in_=d['W2P'][l, m].rearrange('p (a e) -> p a e', e=128))
            g_sb = self.gwp.tile([128, T], BF16, tag='g')
            for n in range(2):
                ps1 = self.acc.tile([128, 512], F32, tag='acc', name='ps1')
                ps2 = self.acc.tile([128, 512], F32, tag='acc', name='ps2')
                for k in range(KD):
                    nc.tensor.matmul(ps1, (w1t[:, k, :]),
                                     (xsT2[:, k, n * 512:(n + 1) * 512]),
                                     start=(k == 0), stop=(k == KD - 1))
                for k in range(KD):
                    nc.tensor.matmul(ps2, (w2t[:, k, :]),
                                     (xsT2[:, k, n * 512:(n + 1) * 512]),
                                     start=(k == 0), stop=(k == KD - 1))
                sil = self.h12.tile([128, T], BF16, tag='h12')
                sl = slice(n * 512, (n + 1) * 512)
                nc.scalar.activation(out=sil[:, sl], in_=ps1, func=AF.Silu,
                                     bias=self.b1s[:, l, m:m + 1])
                nc.vector.scalar_tensor_tensor(
                    out=g_sb[:, sl], in0=ps2, scalar=self.b2s[:, l, m:m + 1],
                    in1=sil[:, sl], op0=ALU.add, op1=ALU.mult)
            nc.sync.dma_start(out=gdram[:, m, :], in_=g_sb)

    def mlp_w3_phase(self, w3n, l, gdram):
        nc = self.nc
        for g in range(2):
            m_ts = [self.amp.tile([128, 768], F32, tag='am', name=f'm{i}')
                    for i in range(4)]
            for n in range(2):
                pss = [self.acc.tile([128, 512], F32, tag='acc', name=f'accw{i}')
                       for i in range(4)]
                for kg in range(4):
                    gt = self.gtp.tile([128, 4, 512], BF16, tag='gt')
                    nc.sync.dma_start(
                        out=gt,
                        in_=gdram[:, kg * 4:(kg + 1) * 4,
                                  g * 512:(g + 1) * 512])
                    for kk in range(4):
                        k = kg * 4 + kk
                        for ti in range(4):
                            nc.tensor.matmul(
                                pss[ti][:, :384],
                                gt[:, kk, ti * 128:(ti + 1) * 128],
                                w3n[n][:, k, :],
                                start=(k == 0), stop=(k == 15))
                for ti in range(4):
                    nc.vector.tensor_copy(
                        out=m_ts[ti][:, 1 + NOFF[n]:1 + NOFF[n] + NCH[n]],
                        in_=pss[ti][:, :NCH[n]])
            for ti in range(4):
                self.residual(self.wy2_s, l, m_ts[ti], g * 4 + ti)

    def final_ln_out(self):
        nc, d = self.nc, self.d
        gb = self.lnfp.tile([128, 2, 767], F32, tag='lnf')
        nc.sync.dma_start(out=gb[:, 0, :], in_=d['lnf_g'].partition_broadcast(128))
        nc.sync.dma_start(out=gb[:, 1, :], in_=d['lnf_b'].partition_broadcast(128))
        for t in range(TM):
            xs = self.tok[:, t, 1:768]
            scr = self.h12.tile([128, T], BF16, tag='h12')
            s2 = self.s2p.tile([128, 4], F32, tag='s2')
            nc.vector.tensor_tensor_reduce(
                out=scr[:, :767], in0=xs, in1=xs, scale=1.0, scalar=0.0,
                op0=ALU.mult, op1=ALU.add, accum_out=s2[:, 0:1])
            nc.vector.reduce_sum(out=s2[:, 1:2], in_=xs, axis=AXX)
            nc.scalar.mul(out=s2[:, 2:3], in_=s2[:, 1:2], mul=1.0 / 767.0)
            nc.vector.tensor_mul(out=s2[:, 3:4], in0=s2[:, 1:2], in1=s2[:, 2:3])
            nc.vector.tensor_sub(out=s2[:, 3:4], in0=s2[:, 0:1], in1=s2[:, 3:4])
            nc.scalar.activation(out=s2[:, 3:4], in_=s2[:, 3:4], func=AF.Ln,
                                 bias=self.lneps[:, 0:1], scale=1.0 / 767.0)
            nc.scalar.activation(out=s2[:, 3:4], in_=s2[:, 3:4], func=AF.Exp,
                                 scale=-0.5)
            res = self.amp.tile([128, 768], F32, tag='am')
            nc.vector.tensor_scalar(out=res[:, 1:768], in0=xs,
                                    scalar1=s2[:, 2:3], scalar2=s2[:, 3:4],
                                    op0=ALU.subtract, op1=ALU.mult)
            nc.vector.tensor_tensor(
                out=res[:, 1:768], in0=res[:, 1:768],
                in1=gb[:, 0, :], op=ALU.mult)
            nc.vector.tensor_tensor(
                out=res[:, 1:768], in0=res[:, 1:768],
                in1=gb[:, 1, :], op=ALU.add)
            scr2 = self.h12.tile([128, T], BF16, tag='h12')
            nc.vector.tensor_tensor_reduce(
                out=scr2[:, :767], in0=res[:, 1:768], in1=res[:, 1:768],
                scale=1.0, scalar=0.0, op0=ALU.mult, op1=ALU.add,
                accum_out=s2[:, 0:1])
            self.sqrt_ln(res[:, 0:1], s2[:, 0:1], C, 0.5)
            b = t // 2
            if t % 2 == 0:
                nc.sync.dma_start(out=d['out'][b * N:b * N + 128, :],
                                  in_=res[:, :768])
            else:
                nc.sync.dma_start(out=d['out'][b * N + 128:(b + 1) * N, :],
                                  in_=res[:N - 128, :768])

    def dump_tok(self):
        nc, d = self.nc, self.d
        for t in range(TM):
            b = t // 2
            if t % 2 == 0:
                nc.sync.dma_start(out=d['out'][b * N:b * N + 128, :],
                                  in_=self.tok[:, t, :])
            else:
                nc.sync.dma_start(out=d['out'][b * N + 128:(b + 1) * N, :],
                                  in_=self.tok[:N - 128, t, :])

    def run(self, n_layers, final_ln):
        nc, d = self.nc, self.d
        self.patch_embed()
        for l in range(n_layers):
            wv = self.wres.tile([128, KD, 768], BF16, tag='wres', name='wv')
            wo = self.wres.tile([128, KD, 768], BF16, tag='wres', name='wo')
            nc.sync.dma_start(
                out=wv, in_=d['WvP'][l].rearrange('p (a e) -> p a e', e=768))
            nc.sync.dma_start(
                out=wo, in_=d['WoP'][l].rearrange('p (a e) -> p a e', e=768))
            w3n = [self.w3res.tile([128, 16, 384], BF16, tag='w3',
                                   name=f'w3n{n}') for n in range(2)]
            for n in range(2):
                nc.sync.dma_start(
                    out=w3n[n],
                    in_=d['W3P'][l, n].rearrange('p (a e) -> p a e', e=384))
            bvs = self.smp.tile([128, 768], F32, tag='bvs')
            nc.sync.dma_start(out=bvs,
                              in_=d['bv'][l:l + 1, :].partition_broadcast(128))
            gdram = self.gdram[l % 2]

            xsT = self.make_xsT()
            qT = self.qkT_phase(xsT, d['WqP'], self.bqs, l)
            kT = self.qkT_phase(xsT, d['WkP'], self.bks, l)
            qrot = self.rope(qT)
            krot = self.rope(kT)
            qt_r = self.head_time(qT, qrot, self.qmask)
            ktn_r = self.head_time(kT, krot, self.kmask)
            vfs = [self.v_img(xsT, wv, bvs, b) for b in range(BC)]
            m_space = self.attention(qT, qrot, kT, krot, qt_r, ktn_r, vfs, l)
            self.wo_phase(m_space, wo, l)
            xsT2 = self.make_xsT()
            self.mlp_h_phase(xsT2, l, gdram)
            self.mlp_w3_phase(w3n, l, gdram)
        if final_ln:
            self.final_ln_out()
        else:
            self.dump_tok()


# ======================================================================
# host entry
# ======================================================================

_CACHE = {}


def _get_program(n_layers=L, final_ln=True):
    key = (n_layers, final_ln)
    if key not in _CACHE:
        _CACHE[key] = build_program(n_layers, final_ln)
    return _CACHE[key]


def kernel(x, cls_s, Wp, ln1_g, ln1_b, Wq, Wk, Wv, Wo, ln2_g, ln2_b,
           W1, W2, W3, wy1, wy2, lnf_g, lnf_b, _n_layers=L, _final_ln=True,
           _trace=False):
    inputs = dict(x=x, cls_s=cls_s, Wp=Wp, ln1_g=ln1_g, ln1_b=ln1_b,
                  Wq=Wq, Wk=Wk, Wv=Wv, Wo=Wo, ln2_g=ln2_g, ln2_b=ln2_b,
                  W1=W1, W2=W2, W3=W3, wy1=wy1, wy2=wy2,
                  lnf_g=lnf_g, lnf_b=lnf_b)
    hp = host_prep(inputs)
    nc = _get_program(_n_layers, _final_ln)
    in_maps = []
    for core in range(NC_CORES):
        m = dict(hp)
        m['xpp'] = core_input(np.asarray(x), core)
        in_maps.append(m)
    res = run_bass_kernel_spmd(nc, in_maps, list(range(NC_CORES)),
                               trace=_trace)
    outs = [res.results[i]['out'].reshape(BC, N, D) for i in range(NC_CORES)]
    full = np.concatenate(outs, 0).astype(np.float32)
    kernel.last_exec_time_ns = res.exec_time_ns
    return full


# revision 34
# speedup vs baseline: 1.5763x; 1.5763x over previous
"""Lorentz-hyperboloid ViT-B (DinoVisionTransformer variant) forward pass on
8 Trainium2 NeuronCores, data-parallel over the batch (4 images / core).

Layout strategy (per core, 4 images, 197 tokens each, img-padded to 256):
  - Residual stream `tok` token-major [1024 (8x128 tiles), 768] f32, col 0 =
    time.
  - LN space-part -> PE-transpose -> xsT feature-major [768, 1024] bf16.
  - qT/kT computed feature-major bf16 (Wq/Wk columns host-permuted so rope
    pair-elements 0/1 form row blocks [0:384) / [384:768)); rope writes the
    even-output rows to a separate `rot` tile (no copy-back).
  - Lorentz scores via two K=32 bf16 matmuls per head on a 256-wide query
    block (one image), plus a K=1 matmul adding the -qt*kt time term
    (replaces per-head DMA broadcasts of qt).
  - Softmax normalization is skipped: the Lorentz projection after the
    attention midpoint is scale-invariant.
  - All sqrt/rsqrt computed as exp(+-0.5*ln(x)) so the whole layer outside
    the MLP runs off one activation table (ln+exp); the MLP uses Silu
    directly -> 2 activation-table swaps per layer.
  - Weights are bf16 and host-prepped into partition-major layouts so each
    weight block loads with a single DMA; Wv/Wo/W3 are SBUF-resident per
    layer, Wq/Wk/W1/W2 stream per 128-column block.
  - SwiGLU gate spilled to DRAM in bf16 (batched, p-major), read back with
    4 DMAs per output block.
"""
import math
import numpy as np
from contextlib import ExitStack

import ml_dtypes
import concourse.bass as bass
import concourse.tile as tile
from concourse import bacc, mybir
from concourse.bass_utils import run_bass_kernel_spmd
from concourse.masks import make_identity

F32 = mybir.dt.float32
F32R = mybir.dt.float32r
BF16 = mybir.dt.bfloat16
AF = mybir.ActivationFunctionType
ALU = mybir.AluOpType
AXX = mybir.AxisListType.X

B, IMG, PS, CIN, D, H, L = 32, 224, 16, 3, 768, 12, 12
HD, DFF, C, EPS = 64, 2048, 1.0, 1e-6
N = 197
NC_CORES = 8
BC = B // NC_CORES
NP = 256                 # padded tokens per image
T = BC * NP              # 1024
TM = T // 128            # 8 token tiles
KD = D // 128            # 6 feature tiles
SCALE = math.sqrt(768.0)
NCH = (384, 383)
NOFF = (0, 384)
SIM_COMPAT = False       # True: avoid Silu (CoreSim lacks it)


def _rope_tables():
    n = IMG // PS
    d4 = HD // 4
    inv = 1.0 / (100.0 ** (np.arange(d4) / d4))
    ang = np.arange(n)[:, None] * inv[None, :]
    ay = np.repeat(ang[:, None, :], n, axis=1)
    ax = np.repeat(ang[None, :, :], n, axis=0)
    a = np.concatenate([ay, ax], -1).reshape(n * n, HD // 2)
    cos = np.concatenate([np.ones((1, HD // 2)), np.cos(a)], 0)
    sin = np.concatenate([np.zeros((1, HD // 2)), np.sin(a)], 0)
    return cos.astype(np.float32), sin.astype(np.float32)


def _pmajor(w, kd):
    """[L, kd*128, E] -> [L, 128, kd*E] partition-major blocks."""
    l, k, e = w.shape
    assert k == kd * 128
    return np.ascontiguousarray(
        w.reshape(l, kd, 128, e).transpose(0, 2, 1, 3).reshape(l, 128, kd * e))


def host_prep(inputs):
    ins = {k: np.ascontiguousarray(np.asarray(v)) for k, v in inputs.items()}
    f32 = np.float32
    bf16 = ml_dtypes.bfloat16

    cos, sin = _rope_tables()
    cosP = np.zeros((128, T), f32)
    sinP = np.zeros((128, T), f32)
    for b in range(BC):
        for rep in range(4):
            cosP[rep * 32:(rep + 1) * 32, b * NP:b * NP + N] = cos.T
            sinP[rep * 32:(rep + 1) * 32, b * NP:b * NP + N] = sin.T

    perm = np.zeros(768, dtype=np.int64)
    for h in range(H):
        for i in range(32):
            perm[h * 32 + i] = h * 64 + 2 * i
            perm[384 + h * 32 + i] = h * 64 + 2 * i + 1

    g1 = ins['ln1_g'][:, :, None]
    b1 = ins['ln1_b']
    g2 = ins['ln2_g'][:, :, None]
    b2 = ins['ln2_b']

    def padrow(w):
        z = np.zeros((w.shape[0], 1, w.shape[2]), f32)
        return np.concatenate([w, z], 1)

    Wq = padrow(g1 * ins['Wq'][:, :, perm])
    Wk = padrow(g1 * ins['Wk'][:, :, perm])
    Wv = padrow(g1 * ins['Wv'])
    bq = np.einsum('ld,lde->le', b1, ins['Wq'][:, :, perm]).astype(f32)
    bk = np.einsum('ld,lde->le', b1, ins['Wk'][:, :, perm]).astype(f32)
    bv = np.einsum('ld,lde->le', b1, ins['Wv']).astype(f32)
    W1 = padrow(g2 * ins['W1'])
    W2 = padrow(g2 * ins['W2'])
    b1m = np.einsum('ld,lde->le', b2, ins['W1']).astype(f32)
    b2m = np.einsum('ld,lde->le', b2, ins['W2']).astype(f32)
    Wo = np.concatenate([ins['Wo'], np.zeros((L, 768, 1), f32)], 2)
    W3 = np.concatenate([ins['W3'], np.zeros((L, 2048, 1), f32)], 2)

    def _mmajor(w):
        # [L, 128, KD*768] -> [L, 6, 128, KD*128] per-output-block layout
        return np.ascontiguousarray(
            w.reshape(L, 128, KD, KD, 128).transpose(0, 3, 1, 2, 4)
            .reshape(L, KD, 128, KD * 128))

    # partition-major bf16 weight layouts (single-DMA loads)
    WqP = _mmajor(_pmajor(Wq, KD)).astype(bf16)
    WkP = _mmajor(_pmajor(Wk, KD)).astype(bf16)
    WvP = _pmajor(Wv, KD).astype(bf16)
    WoP = _pmajor(Wo, KD).astype(bf16)
    # W1/W2: m-major: [L, 16, 128, 6*128]
    W1P = np.ascontiguousarray(
        _pmajor(W1, KD).reshape(L, 128, KD, 16, 128)
        .transpose(0, 3, 1, 2, 4).reshape(L, 16, 128, KD * 128)).astype(bf16)
    W2P = np.ascontiguousarray(
        _pmajor(W2, KD).reshape(L, 128, KD, 16, 128)
        .transpose(0, 3, 1, 2, 4).reshape(L, 16, 128, KD * 128)).astype(bf16)
    # W3: [L, 2, 128, 16*384] halves
    W3p = _pmajor(W3, 16).reshape(L, 128, 16, 768)
    W3P = np.ascontiguousarray(np.stack(
        [W3p[:, :, :, 0:384], W3p[:, :, :, 384:768]], 1)
        .reshape(L, 2, 128, 16 * 384)).astype(bf16)

    Wpr = ins['Wp'].transpose(2, 0, 1, 3).reshape(1024, 767).astype(f32)
    Wpr = np.concatenate([Wpr, np.zeros((1024, 1), f32)], 1)
    WprP = np.ascontiguousarray(
        Wpr.reshape(8, 128, 768).transpose(1, 0, 2)
        .reshape(128, 8 * 768)).astype(bf16)

    cls_s = ins['cls_s']
    cls_vec = np.concatenate(
        [np.sqrt((cls_s ** 2).sum(keepdims=True) + C), cls_s]).astype(f32)

    # head-sum matmul: row k -> output partition 32*(k//32) (band base);
    # all other output partitions stay zero.
    E4 = np.zeros((128, 128), f32)
    for k in range(128):
        E4[k, 32 * (k // 32)] = 1.0

    # per-partition bias tables [128, L*cols]
    bqP = np.ascontiguousarray(
        bq.reshape(L, KD, 128).transpose(2, 0, 1).reshape(128, L * KD))
    bkP = np.ascontiguousarray(
        bk.reshape(L, KD, 128).transpose(2, 0, 1).reshape(128, L * KD))
    b1P = np.ascontiguousarray(
        b1m.reshape(L, 16, 128).transpose(2, 0, 1).reshape(128, L * 16))
    b2P = np.ascontiguousarray(
        b2m.reshape(L, 16, 128).transpose(2, 0, 1).reshape(128, L * 16))

    return {
        'WqP': WqP, 'WkP': WkP, 'WvP': WvP, 'WoP': WoP,
        'W1P': W1P, 'W2P': W2P, 'W3P': W3P,
        'bqP': bqP, 'bkP': bkP, 'b1P': b1P, 'b2P': b2P,
        'bv': bv,
        'WprP': WprP, 'cls': cls_vec.reshape(1, 768),
        'cosP': cosP.astype(bf16), 'sinP': sinP.astype(bf16),
        'E4': E4.astype(bf16),
        'wy1': ins['wy1'].astype(f32).reshape(1, L),
        'wy2': ins['wy2'].astype(f32).reshape(1, L),
        'lnf_g': ins['lnf_g'].astype(f32).reshape(1, 767),
        'lnf_b': ins['lnf_b'].astype(f32).reshape(1, 767),
    }


def core_input(x_full, core):
    f32 = np.float32
    xs = np.asarray(x_full[core * BC:(core + 1) * BC])
    n = IMG // PS
    xp = xs.reshape(BC, 3, n, PS, n, PS).transpose(0, 2, 4, 1, 3, 5)
    xp = xp.reshape(BC, n * n, 3, PS * PS)
    xpp = np.zeros((768, T), f32)
    for b in range(BC):
        cols = b * NP + 1 + np.arange(n * n)
        xpp[:, cols] = xp[b].transpose(1, 2, 0).reshape(768, n * n)
    # partition-major [128, 6, T]
    return np.ascontiguousarray(
        xpp.reshape(6, 128, T).transpose(1, 0, 2)
        .reshape(128, 6 * T)).astype(ml_dtypes.bfloat16)


# ======================================================================
# device program
# ======================================================================

_ACT_TABLES_PATCHED = False


def _patch_act_tables():
    """Restrict the act-table insertion pass to the two tables this kernel
    needs (ln+exp for everything, silu for the MLP) so it can't ping-pong
    between single-function tables.  Table ids stay index-aligned with
    act_info.json because only the func sets are blanked, not the order."""
    global _ACT_TABLES_PATCHED
    if _ACT_TABLES_PATCHED:
        return
    import concourse.hw_specs as hw_specs
    orig = hw_specs.get_activation_tables
    keep = {'sqrt_and_others', 'natural_log_exp_and_others',
            'silu_and_others', 'sigmoid_and_others'}

    def patched(arch):
        return {k: (v if k in keep else set())
                for k, v in orig(arch).items()}

    bacc.get_activation_tables = patched
    _ACT_TABLES_PATCHED = True


def build_program(n_layers=L, final_ln=True):
    _patch_act_tables()
    nc = bacc.Bacc("TRN2", target_bir_lowering=False, debug=False,
                   num_devices=NC_CORES)
    dp = nc.declare_dram_parameter
    d = {}
    for nm, sh, dt in [
            ('xpp', [128, KD * T], BF16),
            ('WqP', [L, KD, 128, KD * 128], BF16),
            ('WkP', [L, KD, 128, KD * 128], BF16),
            ('WvP', [L, 128, KD * 768], BF16), ('WoP', [L, 128, KD * 768], BF16),
            ('W1P', [L, 16, 128, KD * 128], BF16),
            ('W2P', [L, 16, 128, KD * 128], BF16),
            ('W3P', [L, 2, 128, 16 * 384], BF16),
            ('bqP', [128, L * KD], F32), ('bkP', [128, L * KD], F32),
            ('b1P', [128, L * 16], F32), ('b2P', [128, L * 16], F32),
            ('bv', [L, 768], F32),
            ('WprP', [128, 8 * 768], BF16), ('cls', [1, 768], F32),
            ('cosP', [128, T], BF16), ('sinP', [128, T], BF16),
            ('E4', [128, 128], BF16),
            ('wy1', [1, L], F32), ('wy2', [1, L], F32),
            ('lnf_g', [1, 767], F32), ('lnf_b', [1, 767], F32)]:
        d[nm] = dp(nm, sh, dt, isOutput=False).ap()
    d['out'] = dp('out', [BC * N, 768], F32, isOutput=True).ap()

    with tile.TileContext(nc) as tc, ExitStack() as ctx:
        Prog(ctx, tc, d).run(n_layers, final_ln)
    nc.compile()
    return nc


class Prog:
    def __init__(self, ctx, tc, d):
        self.tc, self.nc, self.d = tc, tc.nc, d
        p = lambda name, bufs, space='SBUF': ctx.enter_context(
            tc.tile_pool(name=name, bufs=bufs, space=space))
        self.singles = p('singles', 1)
        self.fm = p('fm', 2)          # xsT / msT / m_space / xsT2  (bf16 12K)
        self.qkp = p('qkp', 2)        # qT / kT bf16
        self.wres = p('wres', 2)      # Wv / Wo resident [128,KD,768] bf16
        self.w3res = p('w3res', 2)    # W3 halves [128,16,384] bf16
        self.w12 = p('w12', 6)        # streamed [128,KD,128] bf16 blocks
        self.gwp = p('gwp', 2)        # g write tiles [128,T] bf16
        self.gtp = p('gtp', 3)        # g read tiles [128,4,512] bf16
        self.ptp = p('ptp', 3)        # exp(scores) [128,2,256] bf16
        self.vfp = p('vfp', 4)        # vf [128,2,12,66] bf16
        self.h12 = p('h12', 3)        # [128,T] bf16 scratch
        self.amp = p('amp', 5)        # a/m token tiles [128,768] f32
        self.smp = p('smp', 1)        # bvs
        self.qtp = p('qtp', 2)        # per-head time tiles [128,3,T] bf16
        self.s2p = p('s2p', 8)        # [128,16] per-partition scalars
        self.dramp = p('dramp', 1, 'DRAM')
        self.acc = p('acc', 4, 'PSUM')    # [128,512] accumulators
        self.mm = p('mm', 2, 'PSUM')      # scores / head-sum psums
        self.tpp = p('tpp', 2, 'PSUM')    # [128,128] transposes

        nc = self.nc
        s = self.singles
        self.tok = s.tile([128, TM, 768], F32)
        self.cos_s = s.tile([128, T], BF16)
        self.sin_s = s.tile([128, T], BF16)
        self.ident = s.tile([128, 128], BF16)
        self.E4_s = s.tile([128, 128], BF16)
        self.qmask = s.tile([128, 1], F32)
        self.kmask = s.tile([128, 1], F32)
        self.wy1_s = s.tile([128, L], F32)
        self.wy2_s = s.tile([128, L], F32)
        self.bqs = s.tile([128, L, KD], F32)
        self.bks = s.tile([128, L, KD], F32)
        self.b1s = s.tile([128, L, 16], F32)
        self.b2s = s.tile([128, L, 16], F32)
        self.eps_s = s.tile([128, 1], F32)
        self.lneps = s.tile([128, 1], F32)
        self.expb = s.tile([128, 1], F32)
        nc.vector.memset(self.lneps, 1e-6)
        nc.vector.memset(self.expb, 2.0 * C / SCALE)
        nc.vector.memset(self.eps_s, EPS)
        nc.vector.memset(self.qmask, 0.0)
        nc.vector.memset(self.kmask, 0.0)
        for r in (0, 32, 64, 96):
            nc.vector.memset(self.qmask[r:r + 1, :], 1.0)
            nc.vector.memset(self.kmask[r:r + 1, :], -1.0)
        nc.sync.dma_start(out=self.cos_s, in_=d['cosP'])
        nc.sync.dma_start(out=self.sin_s, in_=d['sinP'])
        nc.sync.dma_start(out=self.E4_s, in_=d['E4'])
        nc.sync.dma_start(out=self.wy1_s, in_=d['wy1'].partition_broadcast(128))
        nc.sync.dma_start(out=self.wy2_s, in_=d['wy2'].partition_broadcast(128))
        nc.sync.dma_start(out=self.bqs,
                          in_=d['bqP'].rearrange('p (l a) -> p l a', a=KD))
        nc.sync.dma_start(out=self.bks,
                          in_=d['bkP'].rearrange('p (l a) -> p l a', a=KD))
        nc.sync.dma_start(out=self.b1s,
                          in_=d['b1P'].rearrange('p (l a) -> p l a', a=16))
        nc.sync.dma_start(out=self.b2s,
                          in_=d['b2P'].rearrange('p (l a) -> p l a', a=16))
        make_identity(nc, self.ident)
        self.gdram = [self.dramp.tile([128, 16, T], BF16, name=f'gdram{i}')
                      for i in range(2)]

    # ---------------- helpers ----------------
    def pe_T(self, dst, src):
        """PE transpose src [128, w<=128] bf16 -> dst [w, 128] via psum."""
        w = src.shape[-1]
        ps = self.tpp.tile([128, 128], BF16, tag='tp')
        self.nc.tensor.transpose(ps[:w, :], src, self.ident)
        self.nc.vector.tensor_copy(out=dst, in_=ps[:w, :])

    def ln_xsn(self, t):
        """LN (no gain/bias) over space part of tok tile t -> xsn [128,T]
        bf16 (cols 0:767 valid)."""
        nc = self.nc
        xs = self.tok[:, t, 1:768]
        scr = self.h12.tile([128, T], BF16, tag='h12')
        s2 = self.s2p.tile([128, 4], F32, tag='s2')
        nc.scalar.activation(out=scr[:, :767], in_=xs, func=AF.Square,
                             accum_out=s2[:, 0:1])
        nc.vector.reduce_sum(out=s2[:, 1:2], in_=xs, axis=AXX)
        nc.scalar.mul(out=s2[:, 2:3], in_=s2[:, 1:2], mul=1.0 / 767.0)
        nc.vector.tensor_mul(out=s2[:, 3:4], in0=s2[:, 1:2], in1=s2[:, 2:3])
        nc.vector.tensor_sub(out=s2[:, 3:4], in0=s2[:, 0:1], in1=s2[:, 3:4])
        nc.scalar.activation(out=s2[:, 3:4], in_=s2[:, 3:4], func=AF.Sqrt,
                             bias=self.lneps[:, 0:1], scale=1.0 / 767.0)
        nc.vector.reciprocal(out=s2[:, 3:4], in_=s2[:, 3:4])
        xsn = self.h12.tile([128, T], BF16, tag='h12')
        nc.vector.tensor_scalar(out=xsn[:, :767], in0=xs,
                                scalar1=s2[:, 2:3], scalar2=s2[:, 3:4],
                                op0=ALU.subtract, op1=ALU.mult)
        return xsn

    def make_xsT(self):
        xsT = self.fm.tile([128, KD, T], BF16, tag='fm')
        self.nc.vector.memset(xsT[:, 5, :], 0.0)
        for t in range(TM):
            xsn = self.ln_xsn(t)
            for c in range(KD):
                w = min(128, 767 - c * 128)
                self.pe_T(xsT[:w, c, t * 128:(t + 1) * 128],
                          xsn[:, c * 128:c * 128 + w])
        return xsT

    def residual(self, wy_s, l, a_t, t):
        """tok[:,t] = project(tok[:,t] + wy[l] * a) with a = a_t [128,768]
        (space in cols 1:768); computes a's time col first."""
        nc = self.nc
        scr = self.h12.tile([128, T], BF16, tag='h12')
        s2 = self.s2p.tile([128, 4], F32, tag='s2')
        nc.scalar.activation(out=scr[:, :767], in_=a_t[:, 1:768],
                             func=AF.Square, accum_out=s2[:, 0:1])
        nc.scalar.activation(out=a_t[:, 0:1], in_=s2[:, 0:1],
                             func=AF.Sqrt, bias=C)
        tokt = self.tok[:, t, :]
        nc.vector.scalar_tensor_tensor(
            out=tokt, in0=a_t, scalar=wy_s[:, l:l + 1], in1=tokt,
            op0=ALU.mult, op1=ALU.add)
        nc.scalar.activation(out=scr[:, :768], in_=tokt, func=AF.Square,
                             accum_out=s2[:, 1:2])
        nc.vector.tensor_mul(out=s2[:, 2:3], in0=tokt[:, 0:1],
                             in1=tokt[:, 0:1])
        nc.vector.scalar_tensor_tensor(
            out=s2[:, 2:3], in0=s2[:, 2:3], scalar=2.0, in1=s2[:, 1:2],
            op0=ALU.mult, op1=ALU.subtract)        # 2 t^2 - sum = -zz
        nc.vector.tensor_scalar_max(out=s2[:, 2:3], in0=s2[:, 2:3],
                                    scalar1=self.eps_s[:, 0:1])
        nc.scalar.activation(out=s2[:, 2:3], in_=s2[:, 2:3], func=AF.Sqrt)
        nc.vector.reciprocal(out=s2[:, 2:3], in_=s2[:, 2:3])
        nc.vector.tensor_scalar_mul(out=tokt, in0=tokt, scalar1=s2[:, 2:3])

    # ---------------- phases ----------------
    def patch_embed(self):
        nc, d = self.nc, self.d
        wpr = self.fm.tile([128, 8, 768], BF16, tag='fm', name='wpr')
        nc.sync.dma_start(out=wpr,
                          in_=d['WprP'].rearrange('p (a e) -> p a e', e=768))
        xr = d['xpp'].rearrange('p (a t) -> p a t', t=T)
        AT_a = self.qkp.tile([128, 4, T], BF16, tag='qk')
        AT_b = self.qkp.tile([128, 4, T], BF16, tag='qk')
        nc.sync.dma_start(out=AT_a[:, 2:4, :], in_=xr[:, 0:2, :])
        nc.sync.dma_start(out=AT_b, in_=xr[:, 2:6, :])
        for pt in range(2):
            s = self.gtp.tile([128, T], F32, tag='gt')
            t2 = self.gtp.tile([128, T], F32, tag='gt')
            nc.vector.tensor_mul(out=s, in0=AT_a[:, 2 + pt, :],
                                 in1=AT_a[:, 2 + pt, :])
            nc.vector.tensor_mul(out=t2, in0=AT_b[:, pt, :],
                                 in1=AT_b[:, pt, :])
            nc.vector.tensor_add(out=s, in0=s, in1=t2)
            nc.vector.tensor_mul(out=t2, in0=AT_b[:, 2 + pt, :],
                                 in1=AT_b[:, 2 + pt, :])
            nc.vector.tensor_add(out=s, in0=s, in1=t2)
            nc.scalar.activation(out=AT_a[:, pt, :], in_=s, func=AF.Sqrt,
                                 bias=C)
        for g in range(2):
            for n in range(2):
                pss = [self.acc.tile([128, 512], F32, tag='acc', name=f'acc{i}')
                       for i in range(4)]
                for k in range(8):
                    src = AT_a if k < 4 else AT_b
                    for ti in range(4):
                        t = g * 4 + ti
                        nc.tensor.matmul(
                            pss[ti][:, :384],
                            (src[:, k % 4, t * 128:(t + 1) * 128]),
                            (wpr[:, k, NOFF[n]:NOFF[n] + 384]),
                            start=(k == 0), stop=(k == 7))
                for ti in range(4):
                    t = g * 4 + ti
                    nc.vector.tensor_copy(
                        out=self.tok[:, t, 1 + NOFF[n]:1 + NOFF[n] + NCH[n]],
                        in_=pss[ti][:, :NCH[n]])
        for b in range(BC):
            nc.sync.dma_start(out=self.tok[0:1, 2 * b, :], in_=d['cls'])
        for t in range(TM):
            scr = self.h12.tile([128, T], BF16, tag='h12')
            s2 = self.s2p.tile([128, 4], F32, tag='s2')
            nc.scalar.activation(out=scr[:, :767], in_=self.tok[:, t, 1:768],
                                 func=AF.Square, accum_out=s2[:, 0:1])
            nc.scalar.activation(out=self.tok[:, t, 0:1], in_=s2[:, 0:1],
                                 func=AF.Sqrt, bias=C)

    def qkT_phase(self, xsT, wsrc, bias_s, l):
        """qT or kT [128, KD, T] bf16 feature-major = W^T @ xsT (+bias)."""
        nc = self.nc
        dst = self.qkp.tile([128, KD, T], BF16, tag='qk')
        for m in range(KD):
            wt = self.w12.tile([128, KD, 128], BF16, tag='w12')
            nc.sync.dma_start(
                out=wt, in_=wsrc[l, m].rearrange('p (a e) -> p a e', e=128))
            for n in range(2):
                ps = self.acc.tile([128, 512], F32, tag='acc')
                for k in range(KD):
                    nc.tensor.matmul(
                        ps, (wt[:, k, :]),
                        (xsT[:, k, n * 512:(n + 1) * 512]),
                        start=(k == 0), stop=(k == KD - 1))
                nc.scalar.activation(
                    out=dst[:, m, n * 512:(n + 1) * 512], in_=ps,
                    func=AF.Identity, bias=bias_s[:, l, m:m + 1])
        return dst

    def rope(self, zT):
        """Rotate consecutive pairs in place (complex multiply)."""
        nc = self.nc
        for i in range(3):
            z0 = zT[:, i, :]
            z1 = zT[:, i + 3, :]
            t0 = self.h12.tile([128, T], BF16, tag='h12')
            t1 = self.h12.tile([128, T], BF16, tag='h12')
            nc.vector.tensor_mul(out=t0, in0=z0, in1=self.cos_s)
            nc.vector.tensor_mul(out=t1, in0=z1, in1=self.sin_s)
            nc.vector.tensor_sub(out=t0, in0=t0, in1=t1)
            nc.vector.tensor_mul(out=t1, in0=z0, in1=self.sin_s)
            nc.vector.tensor_copy(out=z0, in_=t0)
            nc.vector.tensor_mul(out=t0, in0=z1, in1=self.cos_s)
            nc.vector.tensor_add(out=z1, in0=t1, in1=t0)

    def head_time(self, zT, mask):
        """Per-head Lorentz times as band-aligned tiles [128, 3, T] bf16:
        head h's time row sits at partition 32*(h%4), slot h//4; all other
        partitions zero (negated via mask for the k side)."""
        nc = self.nc
        dst = self.qtp.tile([128, 3, T], BF16, tag='qt')
        for t in range(3):
            ps2 = [self.acc.tile([128, 512], F32, tag='acc', name=f'ht{i}')
                   for i in range(2)]
            for c in (t, t + 3):
                src = zT[:, c, :]
                sq = self.h12.tile([128, T], BF16, tag='h12')
                nc.vector.tensor_mul(out=sq, in0=src, in1=src)
                for cch in range(2):
                    nc.tensor.matmul(
                        ps2[cch], self.E4_s,
                        (sq[:, cch * 512:(cch + 1) * 512]),
                        start=(c == t), stop=(c == t + 3))
            for cch in range(2):
                sl = slice(cch * 512, (cch + 1) * 512)
                nc.scalar.activation(out=dst[:, t, sl], in_=ps2[cch],
                                     func=AF.Sqrt, bias=C)
        nc.vector.tensor_scalar_mul(out=dst, in0=dst, scalar1=mask[:, 0:1])
        return dst

    def v_img(self, xsT, wv, bvs, b):
        """v for image b -> vf_b [128, 2, 12, 66] bf16 (+bias, +time);
        col 65 of each head is padding (never consumed)."""
        nc = self.nc
        vf = self.vfp.tile([128, 2, H, HD + 2], BF16, tag='vf')
        nc.vector.memset(vf[:, :, :, HD + 1:HD + 2], 0.0)
        for n in range(2):
            pss = [self.acc.tile([128, 512], F32, tag='acc', name=f'acc{i}')
                   for i in range(2)]
            for k in range(KD):
                for kt in range(2):
                    t = 2 * b + kt
                    nc.tensor.matmul(
                        pss[kt][:, :384],
                        (xsT[:, k, t * 128:(t + 1) * 128]),
                        (wv[:, k, n * 384:n * 384 + 384]),
                        start=(k == 0), stop=(k == KD - 1))
            for kt in range(2):
                vfv = vf[:, kt, n * 6:(n + 1) * 6, 1:HD + 1]
                psv = pss[kt][:, :384].rearrange('p (h e) -> p h e', e=HD)
                bvv = bvs[:, n * 384:(n + 1) * 384].rearrange(
                    'p (h e) -> p h e', e=HD)
                nc.vector.tensor_tensor(out=vfv, in0=psv, in1=bvv,
                                        op=ALU.add)
                sq = self.h12.tile([128, T], BF16, tag='h12')
                sqv = sq[:, :384].rearrange('p (h e) -> p h e', e=HD)
                nc.vector.tensor_tensor(out=sqv, in0=vfv, in1=vfv,
                                        op=ALU.mult)
                red = self.s2p.tile([128, 16], F32, tag='s2')
                nc.vector.reduce_sum(out=red[:, :6], in_=sqv, axis=AXX)
                # delta = t - 1 = ss / (sqrt(ss + C) + 1)  (C = 1)
                nc.scalar.activation(out=red[:, 8:14], in_=red[:, :6],
                                     func=AF.Sqrt, bias=C)
                nc.scalar.activation(out=red[:, 8:14], in_=red[:, 8:14],
                                     func=AF.Identity, bias=1.0)
                nc.vector.reciprocal(out=red[:, 8:14], in_=red[:, 8:14])
                nc.vector.tensor_mul(out=vf[:, kt, n * 6:(n + 1) * 6, 0],
                                     in0=red[:, :6], in1=red[:, 8:14])
        return vf

    def attention(self, qT, kT, qt_r, ktn_r, vfs, l):
        nc = self.nc
        m_space = self.fm.tile([128, TM, 768], BF16, tag='fm')
        for b in range(BC):
            vf = vfs[b]
            qcol = b * NP
            for half in range(2):
                psA = [self.acc.tile([128, 512], F32, tag='acc', name=f'psA{i}')
                       for i in range(2)]
                for hh in range(6):
                    h = half * 6 + hh
                    r0 = (h * 32) % 128
                    c0 = h // 4
                    P_t = self.ptp.tile([128, 2, NP], BF16, tag='P')
                    for kt in range(2):
                        keys = 128 if kt == 0 else N - 128
                        ps = self.mm.tile([128, 512], F32, tag='mm')
                        kcol = b * NP + kt * 128
                        nc.tensor.matmul(
                            ps[:keys, :NP],
                            (kT[r0:r0 + 32, c0, kcol:kcol + keys]),
                            (qT[r0:r0 + 32, c0, qcol:qcol + NP]),
                            start=True, stop=False, tile_position=(r0, 0))
                        nc.tensor.matmul(
                            ps[:keys, :NP],
                            (kT[r0:r0 + 32, c0 + 3, kcol:kcol + keys]),
                            (qT[r0:r0 + 32, c0 + 3, qcol:qcol + NP]),
                            start=False, stop=False, tile_position=(r0, 0))
                        nc.tensor.matmul(
                            ps[:keys, :NP],
                            (ktn_r[r0:r0 + 32, c0, kcol:kcol + keys]),
                            (qt_r[r0:r0 + 32, c0, qcol:qcol + NP]),
                            start=False, stop=True, tile_position=(r0, 0))
                        nc.scalar.activation(
                            out=P_t[:keys, kt, :], in_=ps[:keys, :NP],
                            func=AF.Exp, bias=self.expb[:keys, 0:1],
                            scale=2.0 / SCALE)
                    for qi in range(2):
                        qn = 128 if qi == 0 else N - 128
                        for kt in range(2):
                            keys = 128 if kt == 0 else N - 128
                            nc.tensor.matmul(
                                psA[qi][:qn, hh * 66:hh * 66 + 66],
                                P_t[:keys, kt, qi * 128:qi * 128 + qn],
                                vf[:keys, kt, h, :],
                                start=(kt == 0), stop=(kt == 1))
                for qi in range(2):
                    qn = 128 if qi == 0 else N - 128
                    psv = psA[qi][:qn, :396].rearrange('p (h e) -> p h e', e=66)
                    mid = self.h12.tile([128, T], BF16, tag='h12')
                    midv = mid[:qn, :396].rearrange('p (h e) -> p h e', e=66)
                    nc.vector.tensor_copy(out=midv, in_=psv)
                    sq = self.h12.tile([128, T], BF16, tag='h12')
                    sqv = sq[:qn, :396].rearrange('p (h e) -> p h e', e=66)
                    nc.vector.tensor_tensor(out=sqv[:, :, 0:65],
                                            in0=midv[:, :, 0:65],
                                            in1=midv[:, :, 0:65], op=ALU.mult)
                    red = self.s2p.tile([128, 16], F32, tag='s2')
                    nc.vector.reduce_sum(out=red[:qn, :6], in_=sqv[:, :, 0:65],
                                         axis=AXX)
                    nc.vector.scalar_tensor_tensor(
                        out=red[:qn, :6], in0=sqv[:, :, 0], scalar=2.0,
                        in1=red[:qn, :6], op0=ALU.mult, op1=ALU.subtract)
                    nc.vector.tensor_scalar_max(
                        out=red[:qn, :6], in0=red[:qn, :6],
                        scalar1=self.eps_s[:qn, 0:1])
                    nc.scalar.activation(out=red[:qn, :6], in_=red[:qn, :6],
                                         func=AF.Ln)
                    nc.scalar.activation(out=red[:qn, :6], in_=red[:qn, :6],
                                         func=AF.Exp, scale=-0.5)
                    mv = m_space[:qn, 2 * b + qi,
                                 half * 384:half * 384 + 384].rearrange(
                                     'p (h e) -> p h e', e=HD)
                    nc.vector.tensor_tensor(
                        out=mv, in0=midv[:, :, 1:65],
                        in1=red[:qn, :6].broadcast_to((qn, 6, HD)),
                        op=ALU.mult)
        return m_space

    def wo_phase(self, m_space, wo, l):
        """msT = m_space^T; a = m_space @ Wo; fused residual-project."""
        nc = self.nc
        msT = self.fm.tile([128, KD, T], BF16, tag='fm')
        for t in range(TM):
            for c in range(KD):
                self.pe_T(msT[:, c, t * 128:(t + 1) * 128],
                          m_space[:, t, c * 128:(c + 1) * 128])
        for g in range(2):
            a_ts = [self.amp.tile([128, 768], F32, tag='am', name=f'a{i}')
                    for i in range(4)]
            for n in range(2):
                pss = [self.acc.tile([128, 512], F32, tag='acc', name=f'acc{i}')
                       for i in range(4)]
                for k in range(KD):
                    for ti in range(4):
                        t = g * 4 + ti
                        nc.tensor.matmul(
                            pss[ti][:, :384],
                            (msT[:, k, t * 128:(t + 1) * 128]),
                            (wo[:, k, NOFF[n]:NOFF[n] + 384]),
                            start=(k == 0), stop=(k == KD - 1))
                for ti in range(4):
                    nc.vector.tensor_copy(
                        out=a_ts[ti][:, 1 + NOFF[n]:1 + NOFF[n] + NCH[n]],
                        in_=pss[ti][:, :NCH[n]])
            for ti in range(4):
                self.residual(self.wy1_s, l, a_ts[ti], g * 4 + ti)

    def mlp_h_phase(self, xsT2, l, gdram):
        nc, d = self.nc, self.d
        for m in range(16):
            w1t = self.w12.tile([128, KD, 128], BF16, tag='w12', name='w1t')
            w2t = self.w12.tile([128, KD, 128], BF16, tag='w12', name='w2t')
            nc.sync.dma_start(
                out=w1t, in_=d['W1P'][l, m].rearrange('p (a e) -> p a e', e=128))
            nc.sync.dma_start(
                out=w2t, in_=d['W2P'][l, m].rearrange('p (a e) -> p a e', e=128))
            g_sb = self.gwp.tile([128, T], BF16, tag='g')
            for n in range(2):
                ps1 = self.acc.tile([128, 512], F32, tag='acc', name='ps1')
                ps2 = self.acc.tile([128, 512], F32, tag='acc', name='ps2')
                for k in range(KD):
                    nc.tensor.matmul(ps1, (w1t[:, k, :]),
                                     (xsT2[:, k, n * 512:(n + 1) * 512]),
                                     start=(k == 0), stop=(k == KD - 1))
                for k in range(KD):
                    nc.tensor.matmul(ps2, (w2t[:, k, :]),
                                     (xsT2[:, k, n * 512:(n + 1) * 512]),
                                     start=(k == 0), stop=(k == KD - 1))
                sil = self.h12.tile([128, T], BF16, tag='h12')
                sl = slice(n * 512, (n + 1) * 512)
                nc.scalar.activation(out=sil[:, sl], in_=ps1, func=AF.Silu,
                                     bias=self.b1s[:, l, m:m + 1])
                nc.vector.scalar_tensor_tensor(
                    out=g_sb[:, sl], in0=ps2, scalar=self.b2s[:, l, m:m + 1],
                    in1=sil[:, sl], op0=ALU.add, op1=ALU.mult)
            nc.sync.dma_start(out=gdram[:, m, :], in_=g_sb)

    def mlp_w3_phase(self, w3n, l, gdram):
        nc = self.nc
        for g in range(2):
            m_ts = [self.amp.tile([128, 768], F32, tag='am', name=f'm{i}')
                    for i in range(4)]
            for n in range(2):
                pss = [self.acc.tile([128, 512], F32, tag='acc', name=f'accw{i}')
                       for i in range(4)]
                for kg in range(4):
                    gt = self.gtp.tile([128, 4, 512], BF16, tag='gt')
                    nc.sync.dma_start(
                        out=gt,
                        in_=gdram[:, kg * 4:(kg + 1) * 4,
                                  g * 512:(g + 1) * 512])
                    for kk in range(4):
                        k = kg * 4 + kk
                        for ti in range(4):
                            nc.tensor.matmul(
                                pss[ti][:, :384],
                                gt[:, kk, ti * 128:(ti + 1) * 128],
                                w3n[n][:, k, :],
                                start=(k == 0), stop=(k == 15))
                for ti in range(4):
                    nc.vector.tensor_copy(
                        out=m_ts[ti][:, 1 + NOFF[n]:1 + NOFF[n] + NCH[n]],
                        in_=pss[ti][:, :NCH[n]])
            for ti in range(4):
                self.residual(self.wy2_s, l, m_ts[ti], g * 4 + ti)

    def final_ln_out(self):
        nc, d = self.nc, self.d
        gb = self.smp.tile([128, 2, 767], F32, tag='lnf')
        nc.sync.dma_start(out=gb[:, 0, :], in_=d['lnf_g'].partition_broadcast(128))
        nc.sync.dma_start(out=gb[:, 1, :], in_=d['lnf_b'].partition_broadcast(128))
        for t in range(TM):
            xs = self.tok[:, t, 1:768]
            scr = self.h12.tile([128, T], BF16, tag='h12')
            s2 = self.s2p.tile([128, 4], F32, tag='s2')
            nc.scalar.activation(out=scr[:, :767], in_=xs, func=AF.Square,
                                 accum_out=s2[:, 0:1])
            nc.vector.reduce_sum(out=s2[:, 1:2], in_=xs, axis=AXX)
            nc.scalar.mul(out=s2[:, 2:3], in_=s2[:, 1:2], mul=1.0 / 767.0)
            nc.vector.tensor_mul(out=s2[:, 3:4], in0=s2[:, 1:2], in1=s2[:, 2:3])
            nc.vector.tensor_sub(out=s2[:, 3:4], in0=s2[:, 0:1], in1=s2[:, 3:4])
            nc.scalar.activation(out=s2[:, 3:4], in_=s2[:, 3:4], func=AF.Sqrt,
                                 bias=self.lneps[:, 0:1], scale=1.0 / 767.0)
            nc.vector.reciprocal(out=s2[:, 3:4], in_=s2[:, 3:4])
            res = self.amp.tile([128, 768], F32, tag='am')
            nc.vector.tensor_scalar(out=res[:, 1:768], in0=xs,
                                    scalar1=s2[:, 2:3], scalar2=s2[:, 3:4],
                                    op0=ALU.subtract, op1=ALU.mult)
            nc.vector.tensor_tensor(
                out=res[:, 1:768], in0=res[:, 1:768],
                in1=gb[:, 0, :], op=ALU.mult)
            nc.vector.tensor_tensor(
                out=res[:, 1:768], in0=res[:, 1:768],
                in1=gb[:, 1, :], op=ALU.add)
            scr2 = self.h12.tile([128, T], BF16, tag='h12')
            nc.scalar.activation(out=scr2[:, :767], in_=res[:, 1:768],
                                 func=AF.Square, accum_out=s2[:, 0:1])
            nc.scalar.activation(out=res[:, 0:1], in_=s2[:, 0:1],
                                 func=AF.Sqrt, bias=C)
            b = t // 2
            if t % 2 == 0:
                nc.sync.dma_start(out=d['out'][b * N:b * N + 128, :],
                                  in_=res[:, :768])
            else:
                nc.sync.dma_start(out=d['out'][b * N + 128:(b + 1) * N, :],
                                  in_=res[:N - 128, :768])

    def dump_tok(self):
        nc, d = self.nc, self.d
        for t in range(TM):
            b = t // 2
            if t % 2 == 0:
                nc.sync.dma_start(out=d['out'][b * N:b * N + 128, :],
                                  in_=self.tok[:, t, :])
            else:
                nc.sync.dma_start(out=d['out'][b * N + 128:(b + 1) * N, :],
                                  in_=self.tok[:N - 128, t, :])

    def run(self, n_layers, final_ln):
        nc, d = self.nc, self.d
        self.patch_embed()
        for l in range(n_layers):
            wv = self.wres.tile([128, KD, 768], BF16, tag='wres', name='wv')
            wo = self.wres.tile([128, KD, 768], BF16, tag='wres', name='wo')
            nc.sync.dma_start(
                out=wv, in_=d['WvP'][l].rearrange('p (a e) -> p a e', e=768))
            nc.sync.dma_start(
                out=wo, in_=d['WoP'][l].rearrange('p (a e) -> p a e', e=768))
            w3n = [self.w3res.tile([128, 16, 384], BF16, tag='w3',
                                   name=f'w3n{n}') for n in range(2)]
            for n in range(2):
                nc.sync.dma_start(
                    out=w3n[n],
                    in_=d['W3P'][l, n].rearrange('p (a e) -> p a e', e=384))
            bvs = self.smp.tile([128, 768], F32, tag='bvs')
            nc.sync.dma_start(out=bvs,
                              in_=d['bv'][l:l + 1, :].partition_broadcast(128))
            gdram = self.gdram[l % 2]

            xsT = self.make_xsT()
            qT = self.qkT_phase(xsT, d['WqP'], self.bqs, l)
            kT = self.qkT_phase(xsT, d['WkP'], self.bks, l)
            self.rope(qT)
            self.rope(kT)
            qt_r = self.head_time(qT, self.qmask)
            ktn_r = self.head_time(kT, self.kmask)
            vfs = [self.v_img(xsT, wv, bvs, b) for b in range(BC)]
            m_space = self.attention(qT, kT, qt_r, ktn_r, vfs, l)
            self.wo_phase(m_space, wo, l)
            xsT2 = self.make_xsT()
            self.mlp_h_phase(xsT2, l, gdram)
            self.mlp_w3_phase(w3n, l, gdram)
        if final_ln:
            self.final_ln_out()
        else:
            self.dump_tok()


# ======================================================================
# host entry
# ======================================================================

_CACHE = {}


def _get_program(n_layers=L, final_ln=True):
    key = (n_layers, final_ln)
    if key not in _CACHE:
        _CACHE[key] = build_program(n_layers, final_ln)
    return _CACHE[key]


def kernel(x, cls_s, Wp, ln1_g, ln1_b, Wq, Wk, Wv, Wo, ln2_g, ln2_b,
           W1, W2, W3, wy1, wy2, lnf_g, lnf_b, _n_layers=L, _final_ln=True,
           _trace=False):
    inputs = dict(x=x, cls_s=cls_s, Wp=Wp, ln1_g=ln1_g, ln1_b=ln1_b,
                  Wq=Wq, Wk=Wk, Wv=Wv, Wo=Wo, ln2_g=ln2_g, ln2_b=ln2_b,
                  W1=W1, W2=W2, W3=W3, wy1=wy1, wy2=wy2,
                  lnf_g=lnf_g, lnf_b=lnf_b)
    hp = host_prep(inputs)
    nc = _get_program(_n_layers, _final_ln)
    in_maps = []
    for core in range(NC_CORES):
        m = dict(hp)
        m['xpp'] = core_input(np.asarray(x), core)
        in_maps.append(m)
    res = run_bass_kernel_spmd(nc, in_maps, list(range(NC_CORES)),
                               trace=_trace)
    outs = [res.results[i]['out'].reshape(BC, N, D) for i in range(NC_CORES)]
    full = np.concatenate(outs, 0).astype(np.float32)
    kernel.last_exec_time_ns = res.exec_time_ns
    return full


# revision 35
# speedup vs baseline: 1.5804x; 1.0026x over previous
"""Lorentz-hyperboloid ViT-B (DinoVisionTransformer variant) forward pass on
8 Trainium2 NeuronCores, data-parallel over the batch (4 images / core).

Layout strategy (per core, 4 images, 197 tokens each, img-padded to 256):
  - Residual stream `tok` token-major [1024 (8x128 tiles), 768] f32, col 0 =
    time.
  - LN space-part -> PE-transpose -> xsT feature-major [768, 1024] bf16.
  - qT/kT computed feature-major bf16 (Wq/Wk columns host-permuted so rope
    pair-elements 0/1 form row blocks [0:384) / [384:768)); rope writes the
    even-output rows to a separate `rot` tile (no copy-back).
  - Lorentz scores via two K=32 bf16 matmuls per head on a 256-wide query
    block (one image), plus a K=1 matmul adding the -qt*kt time term
    (replaces per-head DMA broadcasts of qt).
  - Softmax normalization is skipped: the Lorentz projection after the
    attention midpoint is scale-invariant.
  - All sqrt/rsqrt computed as exp(+-0.5*ln(x)) so the whole layer outside
    the MLP runs off one activation table (ln+exp); the MLP uses Silu
    directly -> 2 activation-table swaps per layer.
  - Weights are bf16 and host-prepped into partition-major layouts so each
    weight block loads with a single DMA; Wv/Wo/W3 are SBUF-resident per
    layer, Wq/Wk/W1/W2 stream per 128-column block.
  - SwiGLU gate spilled to DRAM in bf16 (batched, p-major), read back with
    4 DMAs per output block.
"""
import math
import numpy as np
from contextlib import ExitStack

import ml_dtypes
import concourse.bass as bass
import concourse.tile as tile
from concourse import bacc, mybir
from concourse.bass_utils import run_bass_kernel_spmd
from concourse.masks import make_identity

F32 = mybir.dt.float32
F32R = mybir.dt.float32r
BF16 = mybir.dt.bfloat16
AF = mybir.ActivationFunctionType
ALU = mybir.AluOpType
AXX = mybir.AxisListType.X

B, IMG, PS, CIN, D, H, L = 32, 224, 16, 3, 768, 12, 12
HD, DFF, C, EPS = 64, 2048, 1.0, 1e-6
N = 197
NC_CORES = 8
BC = B // NC_CORES
NP = 256                 # padded tokens per image
T = BC * NP              # 1024
TM = T // 128            # 8 token tiles
KD = D // 128            # 6 feature tiles
SCALE = math.sqrt(768.0)
NCH = (384, 383)
NOFF = (0, 384)
SIM_COMPAT = False       # True: avoid Silu (CoreSim lacks it)


def _rope_tables():
    n = IMG // PS
    d4 = HD // 4
    inv = 1.0 / (100.0 ** (np.arange(d4) / d4))
    ang = np.arange(n)[:, None] * inv[None, :]
    ay = np.repeat(ang[:, None, :], n, axis=1)
    ax = np.repeat(ang[None, :, :], n, axis=0)
    a = np.concatenate([ay, ax], -1).reshape(n * n, HD // 2)
    cos = np.concatenate([np.ones((1, HD // 2)), np.cos(a)], 0)
    sin = np.concatenate([np.zeros((1, HD // 2)), np.sin(a)], 0)
    return cos.astype(np.float32), sin.astype(np.float32)


def _pmajor(w, kd):
    """[L, kd*128, E] -> [L, 128, kd*E] partition-major blocks."""
    l, k, e = w.shape
    assert k == kd * 128
    return np.ascontiguousarray(
        w.reshape(l, kd, 128, e).transpose(0, 2, 1, 3).reshape(l, 128, kd * e))


def host_prep(inputs):
    ins = {k: np.ascontiguousarray(np.asarray(v)) for k, v in inputs.items()}
    f32 = np.float32
    bf16 = ml_dtypes.bfloat16

    cos, sin = _rope_tables()
    cosP = np.zeros((128, T), f32)
    sinP = np.zeros((128, T), f32)
    for b in range(BC):
        for rep in range(4):
            cosP[rep * 32:(rep + 1) * 32, b * NP:b * NP + N] = cos.T
            sinP[rep * 32:(rep + 1) * 32, b * NP:b * NP + N] = sin.T

    perm = np.zeros(768, dtype=np.int64)
    for h in range(H):
        for i in range(32):
            perm[h * 32 + i] = h * 64 + 2 * i
            perm[384 + h * 32 + i] = h * 64 + 2 * i + 1

    g1 = ins['ln1_g'][:, :, None]
    b1 = ins['ln1_b']
    g2 = ins['ln2_g'][:, :, None]
    b2 = ins['ln2_b']

    def padrow(w):
        z = np.zeros((w.shape[0], 1, w.shape[2]), f32)
        return np.concatenate([w, z], 1)

    Wq = padrow(g1 * ins['Wq'][:, :, perm])
    Wk = padrow(g1 * ins['Wk'][:, :, perm])
    Wv = padrow(g1 * ins['Wv'])
    bq = np.einsum('ld,lde->le', b1, ins['Wq'][:, :, perm]).astype(f32)
    bk = np.einsum('ld,lde->le', b1, ins['Wk'][:, :, perm]).astype(f32)
    bv = np.einsum('ld,lde->le', b1, ins['Wv']).astype(f32)
    W1 = padrow(g2 * ins['W1'])
    W2 = padrow(g2 * ins['W2'])
    b1m = np.einsum('ld,lde->le', b2, ins['W1']).astype(f32)
    b2m = np.einsum('ld,lde->le', b2, ins['W2']).astype(f32)
    Wo = np.concatenate([ins['Wo'], np.zeros((L, 768, 1), f32)], 2)
    W3 = np.concatenate([ins['W3'], np.zeros((L, 2048, 1), f32)], 2)

    def _mmajor(w):
        # [L, 128, KD*768] -> [L, 6, 128, KD*128] per-output-block layout
        return np.ascontiguousarray(
            w.reshape(L, 128, KD, KD, 128).transpose(0, 3, 1, 2, 4)
            .reshape(L, KD, 128, KD * 128))

    # partition-major bf16 weight layouts (single-DMA loads)
    WqP = _mmajor(_pmajor(Wq, KD)).astype(bf16)
    WkP = _mmajor(_pmajor(Wk, KD)).astype(bf16)
    WvP = _pmajor(Wv, KD).astype(bf16)
    WoP = _pmajor(Wo, KD).astype(bf16)
    # W1/W2: m-major: [L, 16, 128, 6*128]
    W1P = np.ascontiguousarray(
        _pmajor(W1, KD).reshape(L, 128, KD, 16, 128)
        .transpose(0, 3, 1, 2, 4).reshape(L, 16, 128, KD * 128)).astype(bf16)
    W2P = np.ascontiguousarray(
        _pmajor(W2, KD).reshape(L, 128, KD, 16, 128)
        .transpose(0, 3, 1, 2, 4).reshape(L, 16, 128, KD * 128)).astype(bf16)
    # W3: [L, 2, 128, 16*384] halves
    W3p = _pmajor(W3, 16).reshape(L, 128, 16, 768)
    W3P = np.ascontiguousarray(np.stack(
        [W3p[:, :, :, 0:384], W3p[:, :, :, 384:768]], 1)
        .reshape(L, 2, 128, 16 * 384)).astype(bf16)

    Wpr = ins['Wp'].transpose(2, 0, 1, 3).reshape(1024, 767).astype(f32)
    Wpr = np.concatenate([Wpr, np.zeros((1024, 1), f32)], 1)
    WprP = np.ascontiguousarray(
        Wpr.reshape(8, 128, 768).transpose(1, 0, 2)
        .reshape(128, 8 * 768)).astype(bf16)

    cls_s = ins['cls_s']
    cls_vec = np.concatenate(
        [np.sqrt((cls_s ** 2).sum(keepdims=True) + C), cls_s]).astype(f32)

    # head-sum matmul: row k -> output partition 32*(k//32) (band base);
    # all other output partitions stay zero.
    E4 = np.zeros((128, 128), f32)
    for k in range(128):
        E4[k, 32 * (k // 32)] = 1.0

    # per-partition bias tables [128, L*cols]
    bqP = np.ascontiguousarray(
        bq.reshape(L, KD, 128).transpose(2, 0, 1).reshape(128, L * KD))
    bkP = np.ascontiguousarray(
        bk.reshape(L, KD, 128).transpose(2, 0, 1).reshape(128, L * KD))
    b1P = np.ascontiguousarray(
        b1m.reshape(L, 16, 128).transpose(2, 0, 1).reshape(128, L * 16))
    b2P = np.ascontiguousarray(
        b2m.reshape(L, 16, 128).transpose(2, 0, 1).reshape(128, L * 16))

    return {
        'WqP': WqP, 'WkP': WkP, 'WvP': WvP, 'WoP': WoP,
        'W1P': W1P, 'W2P': W2P, 'W3P': W3P,
        'bqP': bqP, 'bkP': bkP, 'b1P': b1P, 'b2P': b2P,
        'bv': bv,
        'WprP': WprP, 'cls': cls_vec.reshape(1, 768),
        'cosP': cosP.astype(bf16), 'sinP': sinP.astype(bf16),
        'E4': E4.astype(bf16),
        'wy1': ins['wy1'].astype(f32).reshape(1, L),
        'wy2': ins['wy2'].astype(f32).reshape(1, L),
        'lnf_g': ins['lnf_g'].astype(f32).reshape(1, 767),
        'lnf_b': ins['lnf_b'].astype(f32).reshape(1, 767),
    }


def core_input(x_full, core):
    f32 = np.float32
    xs = np.asarray(x_full[core * BC:(core + 1) * BC])
    n = IMG // PS
    xp = xs.reshape(BC, 3, n, PS, n, PS).transpose(0, 2, 4, 1, 3, 5)
    xp = xp.reshape(BC, n * n, 3, PS * PS)
    xpp = np.zeros((768, T), f32)
    for b in range(BC):
        cols = b * NP + 1 + np.arange(n * n)
        xpp[:, cols] = xp[b].transpose(1, 2, 0).reshape(768, n * n)
    # partition-major [128, 6, T]
    return np.ascontiguousarray(
        xpp.reshape(6, 128, T).transpose(1, 0, 2)
        .reshape(128, 6 * T)).astype(ml_dtypes.bfloat16)


# ======================================================================
# device program
# ======================================================================

_ACT_TABLES_PATCHED = False


def _patch_act_tables():
    """Restrict the act-table insertion pass to the two tables this kernel
    needs (ln+exp for everything, silu for the MLP) so it can't ping-pong
    between single-function tables.  Table ids stay index-aligned with
    act_info.json because only the func sets are blanked, not the order."""
    global _ACT_TABLES_PATCHED
    if _ACT_TABLES_PATCHED:
        return
    import concourse.hw_specs as hw_specs
    orig = hw_specs.get_activation_tables
    keep = {'sqrt_and_others', 'natural_log_exp_and_others',
            'silu_and_others', 'sigmoid_and_others'}

    def patched(arch):
        return {k: (v if k in keep else set())
                for k, v in orig(arch).items()}

    bacc.get_activation_tables = patched
    _ACT_TABLES_PATCHED = True


def build_program(n_layers=L, final_ln=True):
    _patch_act_tables()
    nc = bacc.Bacc("TRN2", target_bir_lowering=False, debug=False,
                   num_devices=NC_CORES)
    dp = nc.declare_dram_parameter
    d = {}
    for nm, sh, dt in [
            ('xpp', [128, KD * T], BF16),
            ('WqP', [L, KD, 128, KD * 128], BF16),
            ('WkP', [L, KD, 128, KD * 128], BF16),
            ('WvP', [L, 128, KD * 768], BF16), ('WoP', [L, 128, KD * 768], BF16),
            ('W1P', [L, 16, 128, KD * 128], BF16),
            ('W2P', [L, 16, 128, KD * 128], BF16),
            ('W3P', [L, 2, 128, 16 * 384], BF16),
            ('bqP', [128, L * KD], F32), ('bkP', [128, L * KD], F32),
            ('b1P', [128, L * 16], F32), ('b2P', [128, L * 16], F32),
            ('bv', [L, 768], F32),
            ('WprP', [128, 8 * 768], BF16), ('cls', [1, 768], F32),
            ('cosP', [128, T], BF16), ('sinP', [128, T], BF16),
            ('E4', [128, 128], BF16),
            ('wy1', [1, L], F32), ('wy2', [1, L], F32),
            ('lnf_g', [1, 767], F32), ('lnf_b', [1, 767], F32)]:
        d[nm] = dp(nm, sh, dt, isOutput=False).ap()
    d['out'] = dp('out', [BC * N, 768], F32, isOutput=True).ap()

    with tile.TileContext(nc) as tc, ExitStack() as ctx:
        Prog(ctx, tc, d).run(n_layers, final_ln)
    nc.compile()
    return nc


class Prog:
    def __init__(self, ctx, tc, d):
        self.tc, self.nc, self.d = tc, tc.nc, d
        p = lambda name, bufs, space='SBUF': ctx.enter_context(
            tc.tile_pool(name=name, bufs=bufs, space=space))
        self.singles = p('singles', 1)
        self.fm = p('fm', 2)          # xsT / msT / m_space / xsT2  (bf16 12K)
        self.qkp = p('qkp', 2)        # qT / kT bf16
        self.wres = p('wres', 2)      # Wv / Wo resident [128,KD,768] bf16
        self.w3res = p('w3res', 2)    # W3 halves [128,16,384] bf16
        self.w12 = p('w12', 6)        # streamed [128,KD,128] bf16 blocks
        self.gwp = p('gwp', 2)        # g write tiles [128,T] bf16
        self.gtp = p('gtp', 3)        # g read tiles [128,4,512] bf16
        self.ptp = p('ptp', 3)        # exp(scores) [128,2,256] bf16
        self.vfp = p('vfp', 4)        # vf [128,2,12,66] bf16
        self.h12 = p('h12', 3)        # [128,T] bf16 scratch
        self.amp = p('amp', 5)        # a/m token tiles [128,768] f32
        self.smp = p('smp', 1)        # bvs
        self.qtp = p('qtp', 2)        # per-head time tiles [128,3,T] bf16
        self.s2p = p('s2p', 8)        # [128,16] per-partition scalars
        self.dramp = p('dramp', 1, 'DRAM')
        self.acc = p('acc', 4, 'PSUM')    # [128,512] accumulators
        self.mm = p('mm', 2, 'PSUM')      # scores / head-sum psums
        self.tpp = p('tpp', 2, 'PSUM')    # [128,128] transposes

        nc = self.nc
        s = self.singles
        self.tok = s.tile([128, TM, 768], F32)
        self.cos_s = s.tile([128, T], BF16)
        self.sin_s = s.tile([128, T], BF16)
        self.ident = s.tile([128, 128], BF16)
        self.E4_s = s.tile([128, 128], BF16)
        self.qmask = s.tile([128, 1], F32)
        self.kmask = s.tile([128, 1], F32)
        self.wy1_s = s.tile([128, L], F32)
        self.wy2_s = s.tile([128, L], F32)
        self.bqs = s.tile([128, L, KD], F32)
        self.bks = s.tile([128, L, KD], F32)
        self.b1s = s.tile([128, L, 16], F32)
        self.b2s = s.tile([128, L, 16], F32)
        self.eps_s = s.tile([128, 1], F32)
        self.lneps = s.tile([128, 1], F32)
        self.expb = s.tile([128, 1], F32)
        nc.vector.memset(self.lneps, 1e-6)
        nc.vector.memset(self.expb, 2.0 * C / SCALE)
        nc.vector.memset(self.eps_s, EPS)
        nc.vector.memset(self.qmask, 0.0)
        nc.vector.memset(self.kmask, 0.0)
        for r in (0, 32, 64, 96):
            nc.vector.memset(self.qmask[r:r + 1, :], 1.0)
            nc.vector.memset(self.kmask[r:r + 1, :], -1.0)
        nc.sync.dma_start(out=self.cos_s, in_=d['cosP'])
        nc.sync.dma_start(out=self.sin_s, in_=d['sinP'])
        nc.sync.dma_start(out=self.E4_s, in_=d['E4'])
        nc.sync.dma_start(out=self.wy1_s, in_=d['wy1'].partition_broadcast(128))
        nc.sync.dma_start(out=self.wy2_s, in_=d['wy2'].partition_broadcast(128))
        nc.sync.dma_start(out=self.bqs,
                          in_=d['bqP'].rearrange('p (l a) -> p l a', a=KD))
        nc.sync.dma_start(out=self.bks,
                          in_=d['bkP'].rearrange('p (l a) -> p l a', a=KD))
        nc.sync.dma_start(out=self.b1s,
                          in_=d['b1P'].rearrange('p (l a) -> p l a', a=16))
        nc.sync.dma_start(out=self.b2s,
                          in_=d['b2P'].rearrange('p (l a) -> p l a', a=16))
        make_identity(nc, self.ident)
        self.gdram = [self.dramp.tile([128, 16, T], BF16, name=f'gdram{i}')
                      for i in range(2)]

    # ---------------- helpers ----------------
    def pe_T(self, dst, src):
        """PE transpose src [128, w<=128] bf16 -> dst [w, 128] via psum."""
        w = src.shape[-1]
        ps = self.tpp.tile([128, 128], BF16, tag='tp')
        self.nc.tensor.transpose(ps[:w, :], src, self.ident)
        self.nc.vector.tensor_copy(out=dst, in_=ps[:w, :])

    def ln_xsn(self, t):
        """LN (no gain/bias) over space part of tok tile t -> xsn [128,T]
        bf16 (cols 0:767 valid)."""
        nc = self.nc
        xs = self.tok[:, t, 1:768]
        scr = self.h12.tile([128, T], BF16, tag='h12')
        s2 = self.s2p.tile([128, 4], F32, tag='s2')
        nc.scalar.activation(out=scr[:, :767], in_=xs, func=AF.Square,
                             accum_out=s2[:, 0:1])
        nc.vector.reduce_sum(out=s2[:, 1:2], in_=xs, axis=AXX)
        nc.scalar.mul(out=s2[:, 2:3], in_=s2[:, 1:2], mul=1.0 / 767.0)
        nc.vector.tensor_mul(out=s2[:, 3:4], in0=s2[:, 1:2], in1=s2[:, 2:3])
        nc.vector.tensor_sub(out=s2[:, 3:4], in0=s2[:, 0:1], in1=s2[:, 3:4])
        nc.scalar.activation(out=s2[:, 3:4], in_=s2[:, 3:4], func=AF.Sqrt,
                             bias=self.lneps[:, 0:1], scale=1.0 / 767.0)
        nc.vector.reciprocal(out=s2[:, 3:4], in_=s2[:, 3:4])
        xsn = self.h12.tile([128, T], BF16, tag='h12')
        nc.vector.tensor_scalar(out=xsn[:, :767], in0=xs,
                                scalar1=s2[:, 2:3], scalar2=s2[:, 3:4],
                                op0=ALU.subtract, op1=ALU.mult)
        return xsn

    def make_xsT(self):
        xsT = self.fm.tile([128, KD, T], BF16, tag='fm')
        self.nc.vector.memset(xsT[:, 5, :], 0.0)
        for t in range(TM):
            xsn = self.ln_xsn(t)
            for c in range(KD):
                w = min(128, 767 - c * 128)
                self.pe_T(xsT[:w, c, t * 128:(t + 1) * 128],
                          xsn[:, c * 128:c * 128 + w])
        return xsT

    def residual(self, wy_s, l, a_t, t):
        """tok[:,t] = project(tok[:,t] + wy[l] * a) with a = a_t [128,768]
        (space in cols 1:768); computes a's time col first."""
        nc = self.nc
        scr = self.h12.tile([128, T], BF16, tag='h12')
        s2 = self.s2p.tile([128, 4], F32, tag='s2')
        nc.scalar.activation(out=scr[:, :767], in_=a_t[:, 1:768],
                             func=AF.Square, accum_out=s2[:, 0:1])
        nc.scalar.activation(out=a_t[:, 0:1], in_=s2[:, 0:1],
                             func=AF.Sqrt, bias=C)
        tokt = self.tok[:, t, :]
        nc.vector.scalar_tensor_tensor(
            out=tokt, in0=a_t, scalar=wy_s[:, l:l + 1], in1=tokt,
            op0=ALU.mult, op1=ALU.add)
        nc.scalar.activation(out=scr[:, :768], in_=tokt, func=AF.Square,
                             accum_out=s2[:, 1:2])
        nc.vector.tensor_mul(out=s2[:, 2:3], in0=tokt[:, 0:1],
                             in1=tokt[:, 0:1])
        nc.vector.scalar_tensor_tensor(
            out=s2[:, 2:3], in0=s2[:, 2:3], scalar=2.0, in1=s2[:, 1:2],
            op0=ALU.mult, op1=ALU.subtract)        # 2 t^2 - sum = -zz
        nc.vector.tensor_scalar_max(out=s2[:, 2:3], in0=s2[:, 2:3],
                                    scalar1=self.eps_s[:, 0:1])
        nc.scalar.activation(out=s2[:, 2:3], in_=s2[:, 2:3], func=AF.Sqrt)
        nc.vector.reciprocal(out=s2[:, 2:3], in_=s2[:, 2:3])
        nc.gpsimd.tensor_scalar_mul(out=tokt, in0=tokt, scalar1=s2[:, 2:3])

    # ---------------- phases ----------------
    def patch_embed(self):
        nc, d = self.nc, self.d
        wpr = self.fm.tile([128, 8, 768], BF16, tag='fm', name='wpr')
        nc.sync.dma_start(out=wpr,
                          in_=d['WprP'].rearrange('p (a e) -> p a e', e=768))
        xr = d['xpp'].rearrange('p (a t) -> p a t', t=T)
        AT_a = self.qkp.tile([128, 4, T], BF16, tag='qk')
        AT_b = self.qkp.tile([128, 4, T], BF16, tag='qk')
        nc.sync.dma_start(out=AT_a[:, 2:4, :], in_=xr[:, 0:2, :])
        nc.sync.dma_start(out=AT_b, in_=xr[:, 2:6, :])
        for pt in range(2):
            s = self.gtp.tile([128, T], F32, tag='gt')
            t2 = self.gtp.tile([128, T], F32, tag='gt')
            nc.vector.tensor_mul(out=s, in0=AT_a[:, 2 + pt, :],
                                 in1=AT_a[:, 2 + pt, :])
            nc.vector.tensor_mul(out=t2, in0=AT_b[:, pt, :],
                                 in1=AT_b[:, pt, :])
            nc.vector.tensor_add(out=s, in0=s, in1=t2)
            nc.vector.tensor_mul(out=t2, in0=AT_b[:, 2 + pt, :],
                                 in1=AT_b[:, 2 + pt, :])
            nc.vector.tensor_add(out=s, in0=s, in1=t2)
            nc.scalar.activation(out=AT_a[:, pt, :], in_=s, func=AF.Sqrt,
                                 bias=C)
        for g in range(2):
            for n in range(2):
                pss = [self.acc.tile([128, 512], F32, tag='acc', name=f'acc{i}')
                       for i in range(4)]
                for k in range(8):
                    src = AT_a if k < 4 else AT_b
                    for ti in range(4):
                        t = g * 4 + ti
                        nc.tensor.matmul(
                            pss[ti][:, :384],
                            (src[:, k % 4, t * 128:(t + 1) * 128]),
                            (wpr[:, k, NOFF[n]:NOFF[n] + 384]),
                            start=(k == 0), stop=(k == 7))
                for ti in range(4):
                    t = g * 4 + ti
                    nc.vector.tensor_copy(
                        out=self.tok[:, t, 1 + NOFF[n]:1 + NOFF[n] + NCH[n]],
                        in_=pss[ti][:, :NCH[n]])
        for b in range(BC):
            nc.sync.dma_start(out=self.tok[0:1, 2 * b, :], in_=d['cls'])
        for t in range(TM):
            scr = self.h12.tile([128, T], BF16, tag='h12')
            s2 = self.s2p.tile([128, 4], F32, tag='s2')
            nc.scalar.activation(out=scr[:, :767], in_=self.tok[:, t, 1:768],
                                 func=AF.Square, accum_out=s2[:, 0:1])
            nc.scalar.activation(out=self.tok[:, t, 0:1], in_=s2[:, 0:1],
                                 func=AF.Sqrt, bias=C)

    def qkT_phase(self, xsT, wsrc, bias_s, l):
        """qT or kT [128, KD, T] bf16 feature-major = W^T @ xsT (+bias)."""
        nc = self.nc
        dst = self.qkp.tile([128, KD, T], BF16, tag='qk')
        for m in range(KD):
            wt = self.w12.tile([128, KD, 128], BF16, tag='w12')
            nc.sync.dma_start(
                out=wt, in_=wsrc[l, m].rearrange('p (a e) -> p a e', e=128))
            for n in range(2):
                ps = self.acc.tile([128, 512], F32, tag='acc')
                for k in range(KD):
                    nc.tensor.matmul(
                        ps, (wt[:, k, :]),
                        (xsT[:, k, n * 512:(n + 1) * 512]),
                        start=(k == 0), stop=(k == KD - 1))
                nc.scalar.activation(
                    out=dst[:, m, n * 512:(n + 1) * 512], in_=ps,
                    func=AF.Identity, bias=bias_s[:, l, m:m + 1])
        return dst

    def rope(self, zT):
        """Rotate consecutive pairs in place (complex multiply)."""
        nc = self.nc
        for i in range(3):
            z0 = zT[:, i, :]
            z1 = zT[:, i + 3, :]
            t0 = self.h12.tile([128, T], BF16, tag='h12')
            t1 = self.h12.tile([128, T], BF16, tag='h12')
            nc.vector.tensor_mul(out=t0, in0=z0, in1=self.cos_s)
            nc.gpsimd.tensor_mul(out=t1, in0=z1, in1=self.sin_s)
            nc.vector.tensor_sub(out=t0, in0=t0, in1=t1)
            nc.vector.tensor_mul(out=t1, in0=z0, in1=self.sin_s)
            nc.gpsimd.tensor_copy(out=z0, in_=t0)
            nc.vector.tensor_mul(out=t0, in0=z1, in1=self.cos_s)
            nc.vector.tensor_add(out=z1, in0=t1, in1=t0)

    def head_time(self, zT, mask):
        """Per-head Lorentz times as band-aligned tiles [128, 3, T] bf16:
        head h's time row sits at partition 32*(h%4), slot h//4; all other
        partitions zero (negated via mask for the k side)."""
        nc = self.nc
        dst = self.qtp.tile([128, 3, T], BF16, tag='qt')
        for t in range(3):
            ps2 = [self.acc.tile([128, 512], F32, tag='acc', name=f'ht{i}')
                   for i in range(2)]
            for c in (t, t + 3):
                src = zT[:, c, :]
                sq = self.h12.tile([128, T], BF16, tag='h12')
                nc.vector.tensor_mul(out=sq, in0=src, in1=src)
                for cch in range(2):
                    nc.tensor.matmul(
                        ps2[cch], self.E4_s,
                        (sq[:, cch * 512:(cch + 1) * 512]),
                        start=(c == t), stop=(c == t + 3))
            for cch in range(2):
                sl = slice(cch * 512, (cch + 1) * 512)
                nc.scalar.activation(out=dst[:, t, sl], in_=ps2[cch],
                                     func=AF.Sqrt, bias=C)
        nc.vector.tensor_scalar_mul(out=dst, in0=dst, scalar1=mask[:, 0:1])
        return dst

    def v_img(self, xsT, wv, bvs, b):
        """v for image b -> vf_b [128, 2, 12, 66] bf16 (+bias, +time);
        col 65 of each head is padding (never consumed)."""
        nc = self.nc
        vf = self.vfp.tile([128, 2, H, HD + 2], BF16, tag='vf')
        nc.vector.memset(vf[:, :, :, HD + 1:HD + 2], 0.0)
        for n in range(2):
            pss = [self.acc.tile([128, 512], F32, tag='acc', name=f'acc{i}')
                   for i in range(2)]
            for k in range(KD):
                for kt in range(2):
                    t = 2 * b + kt
                    nc.tensor.matmul(
                        pss[kt][:, :384],
                        (xsT[:, k, t * 128:(t + 1) * 128]),
                        (wv[:, k, n * 384:n * 384 + 384]),
                        start=(k == 0), stop=(k == KD - 1))
            for kt in range(2):
                vfv = vf[:, kt, n * 6:(n + 1) * 6, 1:HD + 1]
                psv = pss[kt][:, :384].rearrange('p (h e) -> p h e', e=HD)
                bvv = bvs[:, n * 384:(n + 1) * 384].rearrange(
                    'p (h e) -> p h e', e=HD)
                nc.vector.tensor_tensor(out=vfv, in0=psv, in1=bvv,
                                        op=ALU.add)
                sq = self.h12.tile([128, T], BF16, tag='h12')
                sqv = sq[:, :384].rearrange('p (h e) -> p h e', e=HD)
                nc.vector.tensor_tensor(out=sqv, in0=vfv, in1=vfv,
                                        op=ALU.mult)
                red = self.s2p.tile([128, 16], F32, tag='s2')
                nc.vector.reduce_sum(out=red[:, :6], in_=sqv, axis=AXX)
                # delta = t - 1 = ss / (sqrt(ss + C) + 1)  (C = 1)
                nc.scalar.activation(out=red[:, 8:14], in_=red[:, :6],
                                     func=AF.Sqrt, bias=C)
                nc.scalar.activation(out=red[:, 8:14], in_=red[:, 8:14],
                                     func=AF.Identity, bias=1.0)
                nc.vector.reciprocal(out=red[:, 8:14], in_=red[:, 8:14])
                nc.vector.tensor_mul(out=vf[:, kt, n * 6:(n + 1) * 6, 0],
                                     in0=red[:, :6], in1=red[:, 8:14])
        return vf

    def attention(self, qT, kT, qt_r, ktn_r, vfs, l):
        nc = self.nc
        m_space = self.fm.tile([128, TM, 768], BF16, tag='fm')
        for b in range(BC):
            vf = vfs[b]
            qcol = b * NP
            for half in range(2):
                psA = [self.acc.tile([128, 512], F32, tag='acc', name=f'psA{i}')
                       for i in range(2)]
                for hh in range(6):
                    h = half * 6 + hh
                    r0 = (h * 32) % 128
                    c0 = h // 4
                    P_t = self.ptp.tile([128, 2, NP], BF16, tag='P')
                    for kt in range(2):
                        keys = 128 if kt == 0 else N - 128
                        ps = self.mm.tile([128, 512], F32, tag='mm')
                        kcol = b * NP + kt * 128
                        nc.tensor.matmul(
                            ps[:keys, :NP],
                            (kT[r0:r0 + 32, c0, kcol:kcol + keys]),
                            (qT[r0:r0 + 32, c0, qcol:qcol + NP]),
                            start=True, stop=False, tile_position=(r0, 0))
                        nc.tensor.matmul(
                            ps[:keys, :NP],
                            (kT[r0:r0 + 32, c0 + 3, kcol:kcol + keys]),
                            (qT[r0:r0 + 32, c0 + 3, qcol:qcol + NP]),
                            start=False, stop=False, tile_position=(r0, 0))
                        nc.tensor.matmul(
                            ps[:keys, :NP],
                            (ktn_r[r0:r0 + 32, c0, kcol:kcol + keys]),
                            (qt_r[r0:r0 + 32, c0, qcol:qcol + NP]),
                            start=False, stop=True, tile_position=(r0, 0))
                        nc.scalar.activation(
                            out=P_t[:keys, kt, :], in_=ps[:keys, :NP],
                            func=AF.Exp, bias=self.expb[:keys, 0:1],
                            scale=2.0 / SCALE)
                    for qi in range(2):
                        qn = 128 if qi == 0 else N - 128
                        for kt in range(2):
                            keys = 128 if kt == 0 else N - 128
                            nc.tensor.matmul(
                                psA[qi][:qn, hh * 66:hh * 66 + 66],
                                P_t[:keys, kt, qi * 128:qi * 128 + qn],
                                vf[:keys, kt, h, :],
                                start=(kt == 0), stop=(kt == 1))
                for qi in range(2):
                    qn = 128 if qi == 0 else N - 128
                    psv = psA[qi][:qn, :396].rearrange('p (h e) -> p h e', e=66)
                    mid = self.h12.tile([128, T], BF16, tag='h12')
                    midv = mid[:qn, :396].rearrange('p (h e) -> p h e', e=66)
                    nc.vector.tensor_copy(out=midv, in_=psv)
                    sq = self.h12.tile([128, T], BF16, tag='h12')
                    sqv = sq[:qn, :396].rearrange('p (h e) -> p h e', e=66)
                    nc.vector.tensor_tensor(out=sqv[:, :, 0:65],
                                            in0=midv[:, :, 0:65],
                                            in1=midv[:, :, 0:65], op=ALU.mult)
                    red = self.s2p.tile([128, 16], F32, tag='s2')
                    nc.vector.reduce_sum(out=red[:qn, :6], in_=sqv[:, :, 0:65],
                                         axis=AXX)
                    nc.vector.scalar_tensor_tensor(
                        out=red[:qn, :6], in0=sqv[:, :, 0], scalar=2.0,
                        in1=red[:qn, :6], op0=ALU.mult, op1=ALU.subtract)
                    nc.vector.tensor_scalar_max(
                        out=red[:qn, :6], in0=red[:qn, :6],
                        scalar1=self.eps_s[:qn, 0:1])
                    nc.scalar.activation(out=red[:qn, :6], in_=red[:qn, :6],
                                         func=AF.Ln)
                    nc.scalar.activation(out=red[:qn, :6], in_=red[:qn, :6],
                                         func=AF.Exp, scale=-0.5)
                    mv = m_space[:qn, 2 * b + qi,
                                 half * 384:half * 384 + 384].rearrange(
                                     'p (h e) -> p h e', e=HD)
                    nc.gpsimd.tensor_tensor(
                        out=mv, in0=midv[:, :, 1:65],
                        in1=red[:qn, :6].broadcast_to((qn, 6, HD)),
                        op=ALU.mult)
        return m_space

    def wo_phase(self, m_space, wo, l, xsT2):
        """msT = m_space^T; a = m_space @ Wo; fused residual-project +
        fused LN2 emission into xsT2."""
        nc = self.nc
        msT = self.fm.tile([128, KD, T], BF16, tag='fm')
        for t in range(TM):
            for c in range(KD):
                self.pe_T(msT[:, c, t * 128:(t + 1) * 128],
                          m_space[:, t, c * 128:(c + 1) * 128])
        for g in range(2):
            a_ts = [self.amp.tile([128, 768], F32, tag='am', name=f'a{i}')
                    for i in range(4)]
            for n in range(2):
                pss = [self.acc.tile([128, 512], F32, tag='acc', name=f'acc{i}')
                       for i in range(4)]
                for k in range(KD):
                    for ti in range(4):
                        t = g * 4 + ti
                        nc.tensor.matmul(
                            pss[ti][:, :384],
                            (msT[:, k, t * 128:(t + 1) * 128]),
                            (wo[:, k, NOFF[n]:NOFF[n] + 384]),
                            start=(k == 0), stop=(k == KD - 1))
                for ti in range(4):
                    nc.vector.tensor_copy(
                        out=a_ts[ti][:, 1 + NOFF[n]:1 + NOFF[n] + NCH[n]],
                        in_=pss[ti][:, :NCH[n]])
            for ti in range(4):
                self.residual(self.wy1_s, l, a_ts[ti], g * 4 + ti,
                              xsT_dst=xsT2)

    def mlp_h_phase(self, xsT2, l, gdram):
        nc, d = self.nc, self.d
        for m in range(16):
            w1t = self.w12.tile([128, KD, 128], BF16, tag='w12', name='w1t')
            w2t = self.w12.tile([128, KD, 128], BF16, tag='w12', name='w2t')
            nc.sync.dma_start(
                out=w1t, in_=d['W1P'][l, m].rearrange('p (a e) -> p a e', e=128))
            nc.sync.dma_start(
                out=w2t, in_=d['W2P'][l, m].rearrange('p (a e) -> p a e', e=128))
            g_sb = self.gwp.tile([128, T], BF16, tag='g')
            for n in range(2):
                ps1 = self.acc.tile([128, 512], F32, tag='acc', name='ps1')
                ps2 = self.acc.tile([128, 512], F32, tag='acc', name='ps2')
                for k in range(KD):
                    nc.tensor.matmul(ps1, (w1t[:, k, :]),
                                     (xsT2[:, k, n * 512:(n + 1) * 512]),
                                     start=(k == 0), stop=(k == KD - 1))
                for k in range(KD):
                    nc.tensor.matmul(ps2, (w2t[:, k, :]),
                                     (xsT2[:, k, n * 512:(n + 1) * 512]),
                                     start=(k == 0), stop=(k == KD - 1))
                sil = self.h12.tile([128, T], BF16, tag='h12')
                sl = slice(n * 512, (n + 1) * 512)
                nc.scalar.activation(out=sil[:, sl], in_=ps1, func=AF.Silu,
                                     bias=self.b1s[:, l, m:m + 1])
                nc.vector.scalar_tensor_tensor(
                    out=g_sb[:, sl], in0=ps2, scalar=self.b2s[:, l, m:m + 1],
                    in1=sil[:, sl], op0=ALU.add, op1=ALU.mult)
            nc.sync.dma_start(out=gdram[:, m, :], in_=g_sb)

    def mlp_w3_phase(self, w3n, l, gdram, xsT_next):
        nc = self.nc
        for g in range(2):
            m_ts = [self.amp.tile([128, 768], F32, tag='am', name=f'm{i}')
                    for i in range(4)]
            for n in range(2):
                pss = [self.acc.tile([128, 512], F32, tag='acc', name=f'accw{i}')
                       for i in range(4)]
                for kg in range(4):
                    gt = self.gtp.tile([128, 4, 512], BF16, tag='gt')
                    nc.sync.dma_start(
                        out=gt,
                        in_=gdram[:, kg * 4:(kg + 1) * 4,
                                  g * 512:(g + 1) * 512])
                    for kk in range(4):
                        k = kg * 4 + kk
                        for ti in range(4):
                            nc.tensor.matmul(
                                pss[ti][:, :384],
                                gt[:, kk, ti * 128:(ti + 1) * 128],
                                w3n[n][:, k, :],
                                start=(k == 0), stop=(k == 15))
                for ti in range(4):
                    nc.vector.tensor_copy(
                        out=m_ts[ti][:, 1 + NOFF[n]:1 + NOFF[n] + NCH[n]],
                        in_=pss[ti][:, :NCH[n]])
            for ti in range(4):
                self.residual(self.wy2_s, l, m_ts[ti], g * 4 + ti,
                              xsT_dst=xsT_next)

    def final_ln_out(self):
        nc, d = self.nc, self.d
        gb = self.smp.tile([128, 2, 767], F32, tag='lnf')
        nc.sync.dma_start(out=gb[:, 0, :], in_=d['lnf_g'].partition_broadcast(128))
        nc.sync.dma_start(out=gb[:, 1, :], in_=d['lnf_b'].partition_broadcast(128))
        for t in range(TM):
            xs = self.tok[:, t, 1:768]
            scr = self.h12.tile([128, T], BF16, tag='h12')
            s2 = self.s2p.tile([128, 4], F32, tag='s2')
            nc.scalar.activation(out=scr[:, :767], in_=xs, func=AF.Square,
                                 accum_out=s2[:, 0:1])
            nc.vector.reduce_sum(out=s2[:, 1:2], in_=xs, axis=AXX)
            nc.scalar.mul(out=s2[:, 2:3], in_=s2[:, 1:2], mul=1.0 / 767.0)
            nc.vector.tensor_mul(out=s2[:, 3:4], in0=s2[:, 1:2], in1=s2[:, 2:3])
            nc.vector.tensor_sub(out=s2[:, 3:4], in0=s2[:, 0:1], in1=s2[:, 3:4])
            nc.scalar.activation(out=s2[:, 3:4], in_=s2[:, 3:4], func=AF.Sqrt,
                                 bias=self.lneps[:, 0:1], scale=1.0 / 767.0)
            nc.vector.reciprocal(out=s2[:, 3:4], in_=s2[:, 3:4])
            res = self.amp.tile([128, 768], F32, tag='am')
            nc.vector.tensor_scalar(out=res[:, 1:768], in0=xs,
                                    scalar1=s2[:, 2:3], scalar2=s2[:, 3:4],
                                    op0=ALU.subtract, op1=ALU.mult)
            nc.vector.tensor_tensor(
                out=res[:, 1:768], in0=res[:, 1:768],
                in1=gb[:, 0, :], op=ALU.mult)
            nc.vector.tensor_tensor(
                out=res[:, 1:768], in0=res[:, 1:768],
                in1=gb[:, 1, :], op=ALU.add)
            scr2 = self.h12.tile([128, T], BF16, tag='h12')
            nc.scalar.activation(out=scr2[:, :767], in_=res[:, 1:768],
                                 func=AF.Square, accum_out=s2[:, 0:1])
            nc.scalar.activation(out=res[:, 0:1], in_=s2[:, 0:1],
                                 func=AF.Sqrt, bias=C)
            b = t // 2
            if t % 2 == 0:
                nc.sync.dma_start(out=d['out'][b * N:b * N + 128, :],
                                  in_=res[:, :768])
            else:
                nc.sync.dma_start(out=d['out'][b * N + 128:(b + 1) * N, :],
                                  in_=res[:N - 128, :768])

    def dump_tok(self):
        nc, d = self.nc, self.d
        for t in range(TM):
            b = t // 2
            if t % 2 == 0:
                nc.sync.dma_start(out=d['out'][b * N:b * N + 128, :],
                                  in_=self.tok[:, t, :])
            else:
                nc.sync.dma_start(out=d['out'][b * N + 128:(b + 1) * N, :],
                                  in_=self.tok[:N - 128, t, :])

    def run(self, n_layers, final_ln):
        nc, d = self.nc, self.d
        self.patch_embed()
        for l in range(n_layers):
            wv = self.wres.tile([128, KD, 768], BF16, tag='wres', name='wv')
            wo = self.wres.tile([128, KD, 768], BF16, tag='wres', name='wo')
            nc.sync.dma_start(
                out=wv, in_=d['WvP'][l].rearrange('p (a e) -> p a e', e=768))
            nc.sync.dma_start(
                out=wo, in_=d['WoP'][l].rearrange('p (a e) -> p a e', e=768))
            w3n = [self.w3res.tile([128, 16, 384], BF16, tag='w3',
                                   name=f'w3n{n}') for n in range(2)]
            for n in range(2):
                nc.sync.dma_start(
                    out=w3n[n],
                    in_=d['W3P'][l, n].rearrange('p (a e) -> p a e', e=384))
            bvs = self.smp.tile([128, 768], F32, tag='bvs')
            nc.sync.dma_start(out=bvs,
                              in_=d['bv'][l:l + 1, :].partition_broadcast(128))
            gdram = self.gdram[l % 2]

            if l == 0:
                xsT = self.make_xsT()
            else:
                xsT = self.xsT_next
            qT = self.qkT_phase(xsT, d['WqP'], self.bqs, l)
            kT = self.qkT_phase(xsT, d['WkP'], self.bks, l)
            self.rope(qT)
            self.rope(kT)
            qt_r = self.head_time(qT, self.qmask)
            ktn_r = self.head_time(kT, self.kmask)
            vfs = [self.v_img(xsT, wv, bvs, b) for b in range(BC)]
            m_space = self.attention(qT, kT, qt_r, ktn_r, vfs, l)
            xsT2 = self.fm.tile([128, KD, T], F16, tag='fm', name='xsT2')
            self.wo_phase(m_space, wo, l, xsT2)
            self.mlp_h_phase(xsT2, l, gdram)
            if l + 1 < n_layers:
                self.xsT_next = self.fm.tile([128, KD, T], F16, tag='fm',
                                             name='xsTn')
            else:
                self.xsT_next = None
            self.mlp_w3_phase(w3n, l, gdram, self.xsT_next)
        if final_ln:
            self.final_ln_out()
        else:
            self.dump_tok()


# ======================================================================
# host entry
# ======================================================================

_CACHE = {}


def _get_program(n_layers=L, final_ln=True):
    key = (n_layers, final_ln)
    if key not in _CACHE:
        _CACHE[key] = build_program(n_layers, final_ln)
    return _CACHE[key]


def kernel(x, cls_s, Wp, ln1_g, ln1_b, Wq, Wk, Wv, Wo, ln2_g, ln2_b,
           W1, W2, W3, wy1, wy2, lnf_g, lnf_b, _n_layers=L, _final_ln=True,
           _trace=False):
    inputs = dict(x=x, cls_s=cls_s, Wp=Wp, ln1_g=ln1_g, ln1_b=ln1_b,
                  Wq=Wq, Wk=Wk, Wv=Wv, Wo=Wo, ln2_g=ln2_g, ln2_b=ln2_b,
                  W1=W1, W2=W2, W3=W3, wy1=wy1, wy2=wy2,
                  lnf_g=lnf_g, lnf_b=lnf_b)
    hp = host_prep(inputs)
    nc = _get_program(_n_layers, _final_ln)
    in_maps = []
    for core in range(NC_CORES):
        m = dict(hp)
        m['xpp'] = core_input(np.asarray(x), core)
        in_maps.append(m)
    res = run_bass_kernel_spmd(nc, in_maps, list(range(NC_CORES)),
                               trace=_trace)
    outs = [res.results[i]['out'].reshape(BC, N, D) for i in range(NC_CORES)]
    full = np.concatenate(outs, 0).astype(np.float32)
    kernel.last_exec_time_ns = res.exec_time_ns
    return full
